# revision 1
# baseline (speedup 1.0000x reference)
"""3-layer GAT on 8 TRN2 NeuronCores via Bass/Tile.

Architecture:
- Nodes dst-sharded 12500/core, re-sorted by in-degree within shard.
- Per-layer node table in each core's DRAM: [100008, 128] bf16 rows
  [feat(64) | el | er | pad], shards of 12501 rows (row 12500 = pad row with
  el = -1e15 so padded slots contribute exp(...)=0).
- Edge gather via InstDMAGatherAnt (int16 idx): 4 windows of 25002 rows,
  per-(tile,window) rectangular slot grids, node-major [128, W, 128].
- Softmax (no max-subtraction; logits are O(1)) on ACT (Lrelu, Exp+accum) and
  DVE (weighted sum via strided-view reduce).
- BN stats via PE ones-matmul + AllReduce; inter-layer AllGather of projected
  shard tables. Layer 1 projects the full (replicated) input locally.
"""
import sys
sys.path.insert(0, "/opt/trn_rl_repo")
import os
import numpy as np
import ml_dtypes

import concourse.bass as bass
import concourse.bacc as bacc
import concourse.tile as tile
import concourse.mybir as mybir
from concourse import bass_utils
from concourse.library_config import mlp as mlp_lib
from concourse.masks import make_identity

N_NODES = 100000
N_EDGES = 1600000
D = 64
N_CORES = 8
SHARD = 12500
SHARD_P = SHARD + 1          # + pad row
N_WIN = 4
WIN_ROWS = 2 * SHARD_P       # 25002 rows per window
TAB_ROWS = N_CORES * SHARD_P # 100008
ROW = 128                    # bf16 elems per table row (256B)
NEG_SLOPE = 0.2
BN_EPS = 1e-5
P = 128
N_TILES = (SHARD + P - 1) // P          # 98 (last tile 84 nodes)
LAST_TILE_N = SHARD - (N_TILES - 1) * P  # 84
CHUNK_TILES = 5
PAD_EL = -1e15
N_LAYERS = int(os.environ.get("GAT_LAYERS", "3"))
NO_COLL = os.environ.get("GAT_NO_COLL", "0") == "1"
RAW_OUT = os.environ.get("GAT_RAW_OUT", "0") == "1"
SIM_SAFE = os.environ.get("GAT_SIM_SAFE", "0") == "1"

f32 = mybir.dt.float32
bf16 = mybir.dt.bfloat16
i16 = mybir.dt.int16


# ---------------------------------------------------------------- host side
def _preprocess(node_weight, src, dst, Ws, als, ars):
    src = np.asarray(src).astype(np.int64)
    dst = np.asarray(dst).astype(np.int64)
    deg = np.bincount(dst, minlength=N_NODES)

    # per-(node, window) incoming-edge counts; window of a src node depends
    # only on its shard (fixed), not the within-shard order.
    src_win0 = (src // SHARD) // 2
    cnt_w = np.zeros((N_NODES, N_WIN), np.int64)
    np.add.at(cnt_w, (dst, src_win0), 1)

    # per-core permutation minimizing per-(tile,window) max: lexsort by
    # (argmax window, -max window count)
    newid = np.empty(N_NODES, np.int64)
    orig_of = np.empty(N_NODES, np.int64)  # new compact (core*SHARD+rank) -> orig
    for c in range(N_CORES):
        orig = np.arange(c * SHARD, (c + 1) * SHARD)
        cw = cnt_w[orig]
        order = orig[np.lexsort((cw.argmax(1), -cw.max(1)))]
        newid[order] = c * SHARD_P + np.arange(SHARD)
        orig_of[c * SHARD: (c + 1) * SHARD] = order

    src_n = newid[src]
    dst_n = newid[dst]
    dst_core = dst // SHARD
    dst_loc = dst_n % SHARD_P  # local rank within shard [0, 12500)

    # group edges per (core, local dst), with per-window counts
    # order edges by (core, dst_loc) for grouping
    win_of_src = src_n // WIN_ROWS

    # per-core structures
    per_core = []
    Wmax = np.zeros((N_TILES, N_WIN), np.int64)
    for c in range(N_CORES):
        m = dst_core == c
        s_c = src_n[m]
        d_c = dst_loc[m]
        w_c = win_of_src[m]
        # sort by (dst_loc, window, src) for deterministic layout
        o = np.lexsort((s_c, w_c, d_c))
        s_c, d_c, w_c = s_c[o], d_c[o], w_c[o]
        # counts[dst_loc, win]
        cnt = np.zeros((SHARD, N_WIN), np.int64)
        np.add.at(cnt, (d_c, w_c), 1)
        per_core.append((s_c, d_c, w_c, cnt))
        # per-tile, per-window max
        for t in range(N_TILES):
            lo, hi = t * P, min((t + 1) * P, SHARD)
            Wmax[t] = np.maximum(Wmax[t], cnt[lo:hi].max(axis=0))

    W_tw = Wmax.astype(np.int64)  # uniform across cores

    # chunk layout
    chunks = []
    t0 = 0
    while t0 < N_TILES:
        chunks.append(list(range(t0, min(t0 + CHUNK_TILES, N_TILES))))
        t0 += CHUNK_TILES

    # per-core idx streams: for each chunk, for each window: int16 idx list
    # (column-major per tile: for t in chunk: for s < W_tw: for p in 0..127)
    idx_streams = []
    call_meta = []  # (chunk_id, win, n_idx, col16_offset) -- shared across cores
    for c in range(N_CORES):
        s_c, d_c, w_c, cnt = per_core[c]
        # slot lists: for each (dst_loc, win) the srcs (window-relative)
        # build offsets: edges sorted by (d, w) so contiguous runs
        # compute run starts per (d, w)
        key = d_c * N_WIN + w_c
        # positions of each (d,w) run
        run_start = np.zeros(SHARD * N_WIN + 1, np.int64)
        np.add.at(run_start, key + 1, 1)
        run_start = np.cumsum(run_start)
        stream = []
        meta = []
        for ci, ch in enumerate(chunks):
            for w in range(N_WIN):
                win_pad = 12500  # window-relative pad row (first shard's pad)
                vals = []
                for t in ch:
                    Wt = int(W_tw[t, w])
                    if Wt == 0:
                        continue
                    n_in_tile = P if t < N_TILES - 1 else LAST_TILE_N
                    block = np.full((Wt, P), win_pad, np.int64)
                    for p in range(n_in_tile):
                        d_l = t * P + p
                        a = run_start[d_l * N_WIN + w]
                        b = run_start[d_l * N_WIN + w + 1]
                        k = b - a
                        if k:
                            block[:k, p] = s_c[a:b] - w * WIN_ROWS
                    vals.append(block.reshape(-1))
                if not vals:
                    if c == 0:
                        meta.append((ci, w, 0, 0))
                    continue
                v = np.concatenate(vals)
                n_idx = v.size  # multiple of 128
                # int16 wrap into 16 partitions, cols n/16, replicate x8
                v16 = v.astype(np.int16).reshape(-1, 16).T  # [16, n/16]
                stream.append(np.tile(v16, (8, 1)))  # [128, n/16]
                if c == 0:
                    meta.append((ci, w, n_idx, 0))
        idx_cat = np.concatenate(stream, axis=1)  # [128, C16]
        idx_streams.append(np.ascontiguousarray(idx_cat))
        if c == 0:
            # fill col16 offsets
            off = 0
            call_meta = []
            k = 0
            for ci, ch in enumerate(chunks):
                for w in range(N_WIN):
                    _, _, n_idx, _ = meta[k]
                    call_meta.append((ci, w, n_idx, off))
                    off += n_idx // 16
                    k += 1

    # layer-1 transposed, permuted, padded input  [64, TAB_ROWS] f32
    nwT = np.zeros((D, TAB_ROWS), np.float32)
    nw = np.asarray(node_weight, np.float32)
    for c in range(N_CORES):
        rows = orig_of[c * SHARD: (c + 1) * SHARD]
        nwT[:, c * SHARD_P: c * SHARD_P + SHARD] = nw[rows].T

    # per-core own-shard transposed input [64, SHARD] (for er matmul path it
    # is just a slice of nwT; pass per-core)
    own_hT = [np.ascontiguousarray(nwT[:, c * SHARD_P: c * SHARD_P + SHARD])
              for c in range(N_CORES)]

    # Wstack per layer [64, 66] = [W | W@al | W@ar]
    wstk = np.concatenate(
        [np.concatenate([Ws[l], (Ws[l] @ als[l])[:, None], (Ws[l] @ ars[l])[:, None]],
                        axis=1)[None] for l in range(3)], axis=0
    ).astype(np.float32)  # [3, 64, 66]

    return dict(
        W_tw=W_tw, chunks=chunks, call_meta=call_meta,
        idx_streams=idx_streams, nwT=nwT, own_hT=own_hT, wstk=wstk,
        orig_of=orig_of,
    )


# ---------------------------------------------------------------- device side
def _build_nc(W_tw, chunks, call_meta, C16):
    nc = bacc.Bacc("TRN2", target_bir_lowering=False, debug=False,
                   num_devices=N_CORES)

    nwT_in = nc.dram_tensor("nwT", [D, TAB_ROWS], f32, kind="ExternalInput")
    ownT_in = nc.dram_tensor("ownT", [D, SHARD], f32, kind="ExternalInput")
    idx_in = nc.dram_tensor("idx", [P, C16], i16, kind="ExternalInput")
    wstk_in = nc.dram_tensor("wstk", [3, D, 66], f32, kind="ExternalInput")
    bnp_in = nc.dram_tensor("bnp", [3, 3, D], f32, kind="ExternalInput")  # b,g,beta
    out_t = nc.dram_tensor("out", [SHARD, D], f32, kind="ExternalOutput")

    rg = [list(range(N_CORES))]
    nc.gpsimd.load_library(mlp_lib)

    with tile.TileContext(nc) as tc:
        with (
            tc.tile_pool(name="const", bufs=1) as constp,
            tc.tile_pool(name="gbuf", bufs=6) as gbuf,
            tc.tile_pool(name="idxb", bufs=4) as idxb,
            tc.tile_pool(name="small", bufs=4) as small,
            tc.tile_pool(name="acc", bufs=1) as accp,
            tc.tile_pool(name="acc2", bufs=3) as accp2,
            tc.tile_pool(name="ps", bufs=2, space="PSUM") as ps,
            tc.tile_pool(name="pstat", bufs=1, space="PSUM") as pstat,
            tc.tile_pool(name="dram", bufs=1, space="DRAM") as dram,
        ):
            ident = constp.tile([P, P], f32)
            make_identity(nc, ident[:])
            ones_col = constp.tile([P, 1], f32)
            nc.vector.memset(ones_col[:], 1.0)

            # weights resident
            wstk_t = constp.tile([D, 3 * 66], f32)
            nc.sync.dma_start(out=wstk_t[:].rearrange("k (l n) -> k l n", n=66), in_=wstk_in[:, :, :].rearrange("l k n -> k l n"))
            bnp_t = constp.tile([P, 9 * D], f32)  # broadcast rows [128, 3*3*64]
            nc.sync.dma_start(out=bnp_t[:], in_=bnp_in[:, :, :].rearrange("l k n -> (l k n)")[None, :].to_broadcast([P, 9 * D]))

            # pad row template [1, 128] bf16: zeros except el=-1e15
            padrow = constp.tile([1, ROW], bf16)
            nc.vector.memset(padrow[:], 0.0)
            nc.vector.memset(padrow[:, 64:66], PAD_EL)

            # er for own shard, per tile column [128, N_TILES] f32
            er_sb = constp.tile([P, N_TILES], f32)
            # out tiles resident [128, N_TILES*64] f32
            out_sb = accp.tile([P, N_TILES * D], f32)

            t0_w = []
            for w in range(N_WIN):
                tbl = dram.tile([WIN_ROWS, ROW], bf16, tag=f"t0w{w}", name=f"t0w{w}")
                t0_w.append(tbl)
            tab1 = dram.tile([TAB_ROWS, ROW], bf16, name="tab1")
            tab2 = dram.tile([TAB_ROWS, ROW], bf16, name="tab2")
            tables = [t0_w,
                      [tab1[w * WIN_ROWS:(w + 1) * WIN_ROWS, :] for w in range(N_WIN)],
                      [tab2[w * WIN_ROWS:(w + 1) * WIN_ROWS, :] for w in range(N_WIN)]]
            ag_tabs = [None, tab1, tab2]
            shard_buf = dram.tile([SHARD_P, ROW], bf16)
            stats_dram_in = dram.tile([D, 2], f32)
            stats_dram_out = dram.tile([D, 2], f32)
            bcast_dram = dram.tile([3, D], f32)

            # ---------------- layer-1: full local projection ----------------
            # own-shard er for layer 1: er = ownT.T @ War1
            for t in range(N_TILES):
                m = P if t < N_TILES - 1 else LAST_TILE_N
                hT = gbuf.tile([D, P], f32, tag="l1e")
                nc.sync.dma_start(out=hT[:, :m], in_=ownT_in[:, t * P: t * P + m])
                pt = ps.tile([P, 1], f32, tag="mm")
                nc.tensor.matmul(out=pt[:m, :], lhsT=hT[:, :m],
                                 rhs=wstk_t[:, 65:66], start=True, stop=True)
                nc.vector.tensor_copy(out=er_sb[:m, t:t + 1], in_=pt[:m, :])

            # process 4 tiles per group: load nwT [64, 512], 4 matmuls,
            # copy to bf16 staging [128, 4*128], strided DMA out.
            GT = 8
            n_groups = (TAB_ROWS + GT * P - 1) // (GT * P)
            for g in range(n_groups):
                col0 = g * GT * P
                ncols = min(GT * P, TAB_ROWS - col0)
                nj = (ncols + P - 1) // P
                hT = gbuf.tile([D, GT * P], f32, tag="l1h")
                nc.sync.dma_start(out=hT[:, :ncols], in_=nwT_in[:, col0:col0 + ncols])
                stage = gbuf.tile([P, GT * 66], bf16, tag="l1s")
                for j in range(nj):
                    m = min(P, ncols - j * P)
                    pt = ps.tile([P, 66], f32, tag="mm")
                    nc.tensor.matmul(
                        out=pt[:m, :], lhsT=hT[:, j * P: j * P + m],
                        rhs=wstk_t[:, 0:66], start=True, stop=True,
                    )
                    nc.scalar.copy(out=stage[:m, j * 66:(j + 1) * 66], in_=pt[:m, :])
                w0 = col0 // WIN_ROWS
                w1 = (col0 + ncols - 1) // WIN_ROWS
                weng = nc.gpsimd if (g % 2 == 0) else nc.sync
                if ncols == GT * P and w0 == w1 and (col0 % WIN_ROWS) % P == 0:
                    r0 = col0 - w0 * WIN_ROWS
                    weng.dma_start(
                        out=tables[0][w0][r0:r0 + ncols, 0:66].rearrange(
                            "(j p) n -> p j n", p=P),
                        in_=stage[:, :].rearrange("p (j n) -> p j n", n=66),
                    )
                else:
                    for j in range(nj):
                        m = min(P, ncols - j * P)
                        rj = col0 + j * P
                        wj = rj // WIN_ROWS
                        if rj + m <= (wj + 1) * WIN_ROWS:
                            weng.dma_start(
                                out=tables[0][wj][rj - wj * WIN_ROWS: rj - wj * WIN_ROWS + m, 0:66],
                                in_=stage[:m, j * 66:(j + 1) * 66],
                            )
                        else:
                            k = (wj + 1) * WIN_ROWS - rj
                            nc.sync.dma_start(
                                out=tables[0][wj][rj - wj * WIN_ROWS: rj - wj * WIN_ROWS + k, 0:66],
                                in_=stage[:k, j * 66:(j + 1) * 66],
                            )
                            nc.sync.dma_start(
                                out=tables[0][wj + 1][0:m - k, 0:66],
                                in_=stage[k:m, j * 66:(j + 1) * 66],
                            )
            # pad rows of table 0 (8 shards)
            for sh in range(N_CORES):
                g_r = sh * SHARD_P + SHARD
                w_r = g_r // WIN_ROWS
                nc.sync.dma_start(
                    out=tables[0][w_r][g_r - w_r * WIN_ROWS: g_r - w_r * WIN_ROWS + 1, :],
                    in_=padrow[:, :],
                )

            # ---------------- per-layer gather + aggregate ----------------
            for l in range(N_LAYERS):
                table = tables[l]  # list of 4 window tiles
                stat_s = pstat.tile([D, 1], f32, tag="stat_s")
                stat_q = pstat.tile([D, 1], f32, tag="stat_q")
                for ci, ch in enumerate(chunks):
                    nch = len(ch)
                    s4c = small.tile([P, nch * N_WIN], f32, tag="s4c")
                    acc4c = accp2.tile([P, nch * N_WIN * D], f32, tag="acc4c")
                    for w in range(N_WIN):
                        meta = call_meta[ci * N_WIN + w]
                        _, _, n_idx, off16 = meta
                        if n_idx == 0:
                            continue
                        it = idxb.tile([P, n_idx // 16], i16, tag="idx")
                        nc.sync.dma_start(out=it[:], in_=idx_in[:, off16: off16 + n_idx // 16])
                        gt = gbuf.tile([P, (n_idx // P) * ROW], bf16, tag="g")
                        nc.gpsimd.dma_gather(
                            out_ap=gt[:].rearrange("p (c r) -> p c r", r=ROW),
                            in_ap=table[w][:, :] if l == 0 else table[w],
                            idxs_ap=it[:, :],
                            num_idxs=n_idx,
                            num_idxs_reg=n_idx,
                            elem_size=ROW,
                            single_packet=False,
                        )
                        o = 0
                        for ti, t in enumerate(ch):
                            Wt = int(W_tw[t, w])
                            if Wt == 0:
                                continue
                            g3 = gt[:].rearrange("p (c r) -> p c r", r=ROW)
                            el_v = g3[:, o:o + Wt, 64:65].rearrange("p w o -> p (w o)")
                            ft_v = g3[:, o:o + Wt, 0:64]
                            e_t = small.tile([P, Wt], f32, tag="e")
                            if SIM_SAFE:
                                nc.scalar.activation(
                                    out=e_t[:], in_=el_v,
                                    func=mybir.ActivationFunctionType.Identity,
                                    bias=er_sb[:, t:t + 1], scale=1.0,
                                )
                                e_s = small.tile([P, Wt], f32, tag="es")
                                nc.vector.tensor_scalar(
                                    out=e_s[:], in0=e_t[:], scalar1=NEG_SLOPE,
                                    scalar2=None, op0=mybir.AluOpType.mult)
                                nc.vector.tensor_tensor(
                                    out=e_t[:], in0=e_t[:], in1=e_s[:],
                                    op=mybir.AluOpType.max)
                            else:
                                nc.scalar.activation(
                                    out=e_t[:], in_=el_v,
                                    func=mybir.ActivationFunctionType.Prelu,
                                    bias=er_sb[:, t:t + 1], scale=1.0,
                                    alpha=NEG_SLOPE,
                                )
                            ex_t = small.tile([P, Wt], f32, tag="x")
                            nc.scalar.activation(
                                out=ex_t[:], in_=e_t[:],
                                func=mybir.ActivationFunctionType.Exp,
                                accum_out=s4c[:, ti * N_WIN + w: ti * N_WIN + w + 1],
                            )
                            wf = small.tile([P, Wt * D], f32, tag="wf")
                            nc.vector.tensor_tensor(
                                out=wf[:].rearrange("p (w d) -> p w d", d=D),
                                in0=ft_v,
                                in1=ex_t[:].unsqueeze(2).to_broadcast([P, Wt, D]),
                                op=mybir.AluOpType.mult,
                            )
                            nc.vector.tensor_reduce(
                                out=acc4c[:, (ti * N_WIN + w) * D:(ti * N_WIN + w + 1) * D],
                                in_=wf[:].rearrange("p (w d) -> p d w", d=D),
                                axis=mybir.AxisListType.X, op=mybir.AluOpType.add,
                            )
                            o += Wt
                    # per tile: combine windows (zero-width windows left uninit:
                    # exclude by summing only active lanes via host-known mask)
                    for ti, t in enumerate(ch):
                        act_ws = [w for w in range(N_WIN) if W_tw[t, w] > 0]
                        base = ti * N_WIN
                        ssum = small.tile([P, 1], f32, tag="ss")
                        if len(act_ws) == N_WIN:
                            nc.vector.tensor_reduce(
                                out=ssum[:], in_=s4c[:, base:base + N_WIN],
                                axis=mybir.AxisListType.X, op=mybir.AluOpType.add)
                        else:
                            nc.vector.tensor_copy(out=ssum[:], in_=s4c[:, base + act_ws[0]: base + act_ws[0] + 1])
                            for w in act_ws[1:]:
                                nc.vector.tensor_tensor(
                                    out=ssum[:], in0=ssum[:],
                                    in1=s4c[:, base + w: base + w + 1],
                                    op=mybir.AluOpType.add)
                        rinv = small.tile([P, 1], f32, tag="ri")
                        nc.vector.reciprocal(out=rinv[:], in_=ssum[:])
                        aggr = small.tile([P, D], f32, tag="ag")
                        if len(act_ws) == N_WIN:
                            nc.vector.tensor_reduce(
                                out=aggr[:],
                                in_=acc4c[:, base * D:(base + N_WIN) * D].rearrange(
                                    "p (w d) -> p d w", d=D),
                                axis=mybir.AxisListType.X, op=mybir.AluOpType.add)
                        else:
                            nc.vector.tensor_copy(
                                out=aggr[:],
                                in_=acc4c[:, (base + act_ws[0]) * D:(base + act_ws[0] + 1) * D])
                            for w in act_ws[1:]:
                                nc.vector.tensor_tensor(
                                    out=aggr[:], in0=aggr[:],
                                    in1=acc4c[:, (base + w) * D:(base + w + 1) * D],
                                    op=mybir.AluOpType.add)
                        ot = out_sb[:, t * D:(t + 1) * D]
                        nc.scalar.activation(
                            out=aggr[:], in_=aggr[:],
                            func=mybir.ActivationFunctionType.Copy,
                            scale=rinv[:, :])
                        nc.vector.tensor_tensor(
                            out=ot, in0=aggr[:], in1=bnp_t[:, (3 * l) * D:(3 * l + 1) * D],
                            op=mybir.AluOpType.add)
                        m = P if t < N_TILES - 1 else LAST_TILE_N
                        sq = small.tile([P, D], f32, tag="sq")
                        nc.scalar.activation(out=sq[:], in_=ot,
                                             func=mybir.ActivationFunctionType.Square)
                        first = (ci == 0 and t == ch[0])
                        last = (t == N_TILES - 1)
                        nc.tensor.matmul(out=stat_s[:, :], lhsT=ot[:m, :],
                                         rhs=ones_col[:m, :],
                                         start=first, stop=last)
                        nc.tensor.matmul(out=stat_q[:, :], lhsT=sq[:m, :],
                                         rhs=ones_col[:m, :],
                                         start=first, stop=last)

                # ---- BN stats all-reduce ----
                stat_sb = small.tile([D, 2], f32, tag="stc")
                nc.vector.tensor_copy(out=stat_sb[:, 0:1], in_=stat_s[:])
                nc.vector.tensor_copy(out=stat_sb[:, 1:2], in_=stat_q[:])
                nc.gpsimd.dma_start(out=stats_dram_in[:], in_=stat_sb[:])
                if not NO_COLL:
                    nc.gpsimd.collective_compute(
                        "AllReduce", mybir.AluOpType.add, replica_groups=rg,
                        ins=[stats_dram_in.opt()], outs=[stats_dram_out.opt()],
                    )
                stat_g = small.tile([D, 2], f32, tag="stg")
                nc.sync.dma_start(out=stat_g[:], in_=(stats_dram_in if NO_COLL else stats_dram_out)[:])
                # mu = s/N ; var = sq/N - mu^2 ; rstd = 1/sqrt(var+eps)
                mu = small.tile([D, 1], f32, tag="mu")
                nc.vector.tensor_scalar(out=mu[:], in0=stat_g[:, 0:1],
                                        scalar1=1.0 / N_NODES, scalar2=None,
                                        op0=mybir.AluOpType.mult)
                musq = small.tile([D, 1], f32, tag="musq")
                nc.scalar.activation(out=musq[:], in_=mu[:],
                                     func=mybir.ActivationFunctionType.Square)
                var = small.tile([D, 1], f32, tag="var")
                nc.vector.tensor_scalar(out=var[:], in0=stat_g[:, 1:2],
                                        scalar1=1.0 / N_NODES, scalar2=None,
                                        op0=mybir.AluOpType.mult)
                nc.vector.tensor_tensor(out=var[:], in0=var[:], in1=musq[:],
                                        op=mybir.AluOpType.subtract)
                nc.vector.tensor_scalar(out=var[:], in0=var[:], scalar1=BN_EPS,
                                        scalar2=None, op0=mybir.AluOpType.add)
                sd = small.tile([D, 1], f32, tag="sd")
                nc.scalar.activation(out=sd[:], in_=var[:],
                                     func=mybir.ActivationFunctionType.Sqrt)
                rstd = small.tile([D, 1], f32, tag="rstd")
                nc.vector.reciprocal(out=rstd[:], in_=sd[:])
                # column vectors for dim-major BN: g/beta as [D,1]
                gcol = small.tile([D, 1], f32, tag="gc")
                nc.sync.dma_start(out=gcol[:], in_=bnp_in[l, 1, :][:, None])
                bcol = small.tile([D, 1], f32, tag="bc")
                nc.sync.dma_start(out=bcol[:], in_=bnp_in[l, 2, :][:, None])
                grs = small.tile([D, 1], f32, tag="grs")
                nc.vector.tensor_tensor(out=grs[:], in0=gcol[:], in1=rstd[:],
                                        op=mybir.AluOpType.mult)
                negmu = small.tile([D, 1], f32, tag="nmu")
                nc.vector.tensor_scalar(out=negmu[:], in0=mu[:], scalar1=-1.0,
                                        scalar2=None, op0=mybir.AluOpType.mult)

                if l < N_LAYERS - 1:
                    # pass 2: transpose out tiles, BN+ELU, project, write shard_buf
                    for t in range(N_TILES):
                        m = P if t < N_TILES - 1 else LAST_TILE_N
                        ot = out_sb[:, t * D:(t + 1) * D]
                        pT = ps.tile([D, P], f32, tag="pT")
                        nc.tensor.transpose(out=pT[:, :m], in_=ot[:m, :], identity=ident[:m, :m])
                        z = small.tile([D, P], f32, tag="z")
                        # z = (x - mu) * grs + beta
                        nc.vector.tensor_scalar(
                            out=z[:, :m], in0=pT[:, :m], scalar1=negmu[:, :],
                            scalar2=grs[:, :], op0=mybir.AluOpType.add,
                            op1=mybir.AluOpType.mult)
                        nc.vector.tensor_scalar(
                            out=z[:, :m], in0=z[:, :m], scalar1=bcol[:, :],
                            scalar2=None, op0=mybir.AluOpType.add)
                        # ELU: relu(z) + min(exp(z)-1, 0)
                        ez = small.tile([D, P], f32, tag="ez")
                        nc.scalar.activation(out=ez[:, :m], in_=z[:, :m],
                                             func=mybir.ActivationFunctionType.Exp)
                        nc.vector.tensor_scalar(
                            out=ez[:, :m], in0=ez[:, :m], scalar1=-1.0, scalar2=0.0,
                            op0=mybir.AluOpType.add, op1=mybir.AluOpType.min)
                        nc.vector.tensor_scalar(
                            out=z[:, :m], in0=z[:, :m], scalar1=0.0, scalar2=None,
                            op0=mybir.AluOpType.max)
                        h2 = small.tile([D, P], f32, tag="h2")
                        nc.vector.tensor_tensor(out=h2[:, :m], in0=z[:, :m],
                                                in1=ez[:, :m], op=mybir.AluOpType.add)
                        # project with next layer weights
                        pj = ps.tile([P, 66], f32, tag="mm")
                        nc.tensor.matmul(out=pj[:m, :], lhsT=h2[:, :m],
                                         rhs=wstk_t[:, (l + 1) * 66:(l + 2) * 66],
                                         start=True, stop=True)
                        stg = small.tile([P, 66], bf16, tag="stg2")
                        nc.scalar.copy(out=stg[:m, :], in_=pj[:m, :])
                        nc.gpsimd.dma_start(out=shard_buf[t * P: t * P + m, 0:66],
                                          in_=stg[:m, :])
                        nc.vector.tensor_copy(out=er_sb[:m, t:t + 1], in_=pj[:m, 65:66])
                    nc.sync.dma_start(out=shard_buf[SHARD:SHARD + 1, :], in_=padrow[:, :])
                    if not NO_COLL:
                        nc.gpsimd.collective_compute(
                            "AllGather", mybir.AluOpType.bypass, replica_groups=rg,
                            ins=[shard_buf.opt()], outs=[ag_tabs[l + 1].opt()],
                        )
                elif RAW_OUT:
                    for t in range(N_TILES):
                        m = P if t < N_TILES - 1 else LAST_TILE_N
                        nc.sync.dma_start(out=out_t[t * P:t * P + m, :],
                                          in_=out_sb[:m, t * D:(t + 1) * D])
                else:
                    # final BN in node-major; need row-broadcast vectors
                    nc.gpsimd.dma_start(out=bcast_dram[0, :], in_=negmu[:, 0])
                    nc.gpsimd.dma_start(out=bcast_dram[1, :], in_=grs[:, 0])
                    nc.gpsimd.dma_start(out=bcast_dram[2, :], in_=bcol[:, 0])
                    brow = small.tile([P, 3 * D], f32, tag="brow")
                    nc.sync.dma_start(
                        out=brow[:],
                        in_=bcast_dram[:, :].rearrange("a b -> (a b)")[None, :].to_broadcast([P, 3 * D]))
                    for t in range(N_TILES):
                        m = P if t < N_TILES - 1 else LAST_TILE_N
                        ot = out_sb[:, t * D:(t + 1) * D]
                        y = small.tile([P, D], f32, tag="y")
                        nc.vector.tensor_tensor(out=y[:m, :], in0=ot[:m, :],
                                                in1=brow[:m, 0:D], op=mybir.AluOpType.add)
                        nc.vector.tensor_tensor(out=y[:m, :], in0=y[:m, :],
                                                in1=brow[:m, D:2 * D], op=mybir.AluOpType.mult)
                        nc.vector.tensor_tensor(out=y[:m, :], in0=y[:m, :],
                                                in1=brow[:m, 2 * D:3 * D], op=mybir.AluOpType.add)
                        nc.sync.dma_start(out=out_t[t * P:t * P + m, :], in_=y[:m, :])

    nc.compile()
    return nc


_CACHE = {}


def kernel(node_weight, edge_weight, src, dst,
           W1, al1, ar1, b1, g1, beta1,
           W2, al2, ar2, b2, g2, beta2,
           W3, al3, ar3, b3, g3, beta3):
    Ws = [np.asarray(W1, np.float32), np.asarray(W2, np.float32), np.asarray(W3, np.float32)]
    als = [np.asarray(al1, np.float32), np.asarray(al2, np.float32), np.asarray(al3, np.float32)]
    ars = [np.asarray(ar1, np.float32), np.asarray(ar2, np.float32), np.asarray(ar3, np.float32)]
    pre = _preprocess(node_weight, src, dst, Ws, als, ars)

    C16 = pre["idx_streams"][0].shape[1]
    key = ("nc", C16, N_LAYERS, NO_COLL, RAW_OUT, tuple(pre["W_tw"].reshape(-1).tolist()))
    if key not in _CACHE:
        _CACHE[key] = _build_nc(pre["W_tw"], pre["chunks"], pre["call_meta"], C16)
    nc = _CACHE[key]

    bnp = np.stack([
        np.stack([np.asarray(b, np.float32), np.asarray(g, np.float32),
                  np.asarray(be, np.float32)])
        for b, g, be in ((b1, g1, beta1), (b2, g2, beta2), (b3, g3, beta3))
    ])  # [3, 3, 64]

    in_maps = []
    for c in range(N_CORES):
        in_maps.append({
            "nwT": pre["nwT"],
            "ownT": pre["own_hT"][c],
            "idx": pre["idx_streams"][c],
            "wstk": pre["wstk"],
            "bnp": bnp,
        })
    res = bass_utils.run_bass_kernel_spmd(nc, in_maps, core_ids=list(range(N_CORES)))

    out = np.empty((N_NODES, D), np.float32)
    for c in range(N_CORES):
        rows = pre["orig_of"][c * SHARD: (c + 1) * SHARD]
        out[rows] = res.results[c]["out"]
    return out



# revision 57
# speedup vs baseline: 1.3380x; 1.3380x over previous
"""3-layer GAT on 8 TRN2 NeuronCores via Bass/Tile.

Architecture (v2):
- Nodes dst-sharded 12500/core, re-sorted by (argmax window, -max window count).
- Per-layer node table in each core's DRAM: [100008, 128] bf16 rows
  [feat(64) | el | er | pad], 4 windows of 25002 rows (row 12500 of each
  window = pad row with el = -1e15 so padded slots contribute exp(...)=0).
  The layer-1 table (a pure linear fold of the inputs) is precomputed on the
  host and passed as an input, so layer-1 gathers start immediately.
- Edge gather via InstDMAGatherAnt (int16 idx) per (chunk, window) with
  chunk-uniform width W_cw: grid [P, T, W, 128] bf16. Chunk boundaries chosen
  by DP to minimize padded slots + fixed per-chunk overhead.
- Per (chunk, window): DVE logit-add (+er broadcast), ACT Prelu, ACT Exp,
  DVE exsum-reduce; ACT expands ex to a dense fp16 [P,T,W,64] so the weighted
  mult runs in the 2x 16-bit DVE mode; DVE reduce to fp16 window partials;
  per-chunk window combine + 1/(sum+eps) scale. The GAT bias b provably
  cancels through BatchNorm and is dropped; eps makes isolated nodes exact 0.
- BN stats incrementally per chunk (ACT square + PSUM-chained PE ones-matmuls)
  + AllReduce; inter-layer transform in groups of 4 tiles (PE transpose, fused
  BN, ELU with exp-bias fold, projection); AllGather of projected shard
  tables. Engine queues: Pool = gathers only, SP = idx/outputs, ACT = small
  DMAs + activations, so no head-of-line blocking on the gather path.
"""
import sys
sys.path.insert(0, "/opt/trn_rl_repo")
import os
import numpy as np
import ml_dtypes

import concourse.bass as bass
import concourse.bacc as bacc
import concourse.tile as tile
import concourse.mybir as mybir
from concourse import bass_utils
from concourse.library_config import mlp as mlp_lib
from concourse.masks import make_identity

N_NODES = 100000
N_EDGES = 1600000
D = 64
N_CORES = 8
SHARD = 12500
SHARD_P = SHARD + 1          # + pad row
N_WIN = 4
WIN_ROWS = 2 * SHARD_P       # 25002 rows per window
TAB_ROWS = N_CORES * SHARD_P # 100008
ROW = 128                    # bf16 elems per table row (256B)
NEG_SLOPE = 0.2
BN_EPS = 1e-5
P = 128
N_TILES = (SHARD + P - 1) // P          # 98 (last tile 84 nodes)
LAST_TILE_N = SHARD - (N_TILES - 1) * P  # 84
PAD_EL = -1e15
WIN_PAD = 12500              # window-relative pad row
N_LAYERS = int(os.environ.get("GAT_LAYERS", "3"))
NO_COLL = os.environ.get("GAT_NO_COLL", "0") == "1"
# DP chunking knobs
CHUNK_FIXED_SLOTS = 2000     # fixed per-chunk cost in slot-equivalents
CHUNK_MAX_TILES = 14
CHUNK_MAX_COLS = 64          # T * max_w W_cw  (bounds SBUF grid size)

f32 = mybir.dt.float32
bf16 = mybir.dt.bfloat16
i16 = mybir.dt.int16
f16 = mybir.dt.float16


# ---------------------------------------------------------------- host side
def _chunk_dp(W_t):
    """Pick chunk boundaries minimizing sum(T*sum_w max W) * P + fixed."""
    n = W_t.shape[0]
    INF = float("inf")
    dp = [INF] * (n + 1)
    prv = [0] * (n + 1)
    dp[0] = 0.0
    for j in range(1, n + 1):
        for i in range(max(0, j - CHUNK_MAX_TILES), j):
            T = j - i
            Wm = W_t[i:j].max(axis=0)
            if T * Wm.max() > CHUNK_MAX_COLS:
                continue
            c = dp[i] + T * Wm.sum() * P + CHUNK_FIXED_SLOTS
            if c < dp[j]:
                dp[j] = c
                prv[j] = i
    bounds = []
    j = n
    while j > 0:
        i = prv[j]
        bounds.append((i, j))
        j = i
    bounds.reverse()
    return [list(range(i, j)) for i, j in bounds]


def _preprocess(node_weight, src, dst, Ws, als, ars):
    src = np.asarray(src).astype(np.int64)
    dst = np.asarray(dst).astype(np.int64)

    # per-(node, window) incoming-edge counts; window of a src node depends
    # only on its shard (fixed), not the within-shard order.
    src_win = (src // SHARD) // 2
    cnt_w = np.zeros((N_NODES, N_WIN), np.int64)
    np.add.at(cnt_w, (dst, src_win), 1)

    newid = np.empty(N_NODES, np.int64)
    orig_of = np.empty(N_NODES, np.int64)  # compact (core*SHARD+rank) -> orig
    for c in range(N_CORES):
        orig = np.arange(c * SHARD, (c + 1) * SHARD)
        cw = cnt_w[orig]
        order = orig[np.lexsort((cw.argmax(1), -cw.max(1)))]
        newid[order] = c * SHARD_P + np.arange(SHARD)
        orig_of[c * SHARD: (c + 1) * SHARD] = order

    src_n = newid[src]
    dst_core = dst // SHARD
    dst_loc = newid[dst] % SHARD_P       # local rank within shard [0, 12500)
    win_of_src = src_n // WIN_ROWS

    # per-core grouped edges + per-tile window maxima (max over cores)
    per_core = []
    W_t = np.zeros((N_TILES, N_WIN), np.int64)
    for c in range(N_CORES):
        m = dst_core == c
        s_c, d_c, w_c = src_n[m], dst_loc[m], win_of_src[m]
        o = np.lexsort((s_c, w_c, d_c))
        s_c, d_c, w_c = s_c[o], d_c[o], w_c[o]
        cnt = np.zeros((SHARD, N_WIN), np.int64)
        np.add.at(cnt, (d_c, w_c), 1)
        per_core.append((s_c, d_c, w_c, cnt))
        cm = cnt.reshape(-1)[: (N_TILES - 1) * P * N_WIN].reshape(N_TILES - 1, P, N_WIN).max(axis=1)
        W_t[: N_TILES - 1] = np.maximum(W_t[: N_TILES - 1], cm)
        W_t[N_TILES - 1] = np.maximum(
            W_t[N_TILES - 1], cnt[(N_TILES - 1) * P:].max(axis=0))

    chunks = _chunk_dp(W_t)
    # per (chunk, window) uniform width + call metadata (uniform across cores)
    W_cw = [[int(W_t[ch].max(axis=0)[w]) for w in range(N_WIN)] for ch in chunks]
    call_meta = []  # list per chunk: dict(cols0, len16, calls=[(w, W, coloff, n_idx)])
    off16 = 0
    col = 0
    for ci, ch in enumerate(chunks):
        T = len(ch)
        calls = []
        cols0 = col
        for w in range(N_WIN):
            W = W_cw[ci][w]
            if W == 0:
                continue
            calls.append((w, W, col - cols0, T * W * P))
            col += T * W
        call_meta.append(dict(cols0=cols0, ncols=col - cols0, off16=off16,
                              calls=calls, tiles=ch))
        off16 += (col - cols0) * P // 16
    total_cols = col

    # per-core idx streams (vectorized fill)
    tile_of = np.zeros(N_TILES, np.int64)
    tloc_of = np.zeros(N_TILES, np.int64)
    chunk_of = np.zeros(N_TILES, np.int64)
    for ci, ch in enumerate(chunks):
        for k, t in enumerate(ch):
            chunk_of[t] = ci
            tloc_of[t] = k
    # column offset of (chunk, window) in global stream
    colbase = np.zeros((len(chunks), N_WIN), np.int64)
    for ci, meta in enumerate(call_meta):
        for (w, W, coloff, _n) in meta["calls"]:
            colbase[ci, w] = meta["cols0"] + coloff
    Wmat = np.array([[W_cw[ci][w] for w in range(N_WIN)]
                     for ci in range(len(chunks))], np.int64)

    idx_streams = []
    for c in range(N_CORES):
        s_c, d_c, w_c, cnt = per_core[c]
        key = d_c * N_WIN + w_c
        run_start = np.zeros(SHARD * N_WIN + 1, np.int64)
        np.add.at(run_start, key + 1, 1)
        run_start = np.cumsum(run_start)
        rank = np.arange(len(s_c)) - run_start[key]
        t = d_c // P
        p = d_c % P
        ci = chunk_of[t]
        colg = colbase[ci, w_c] + tloc_of[t] * Wmat[ci, w_c] + rank
        stream = np.full(total_cols * P, WIN_PAD, np.int64)
        stream[colg * P + p] = s_c - w_c * WIN_ROWS
        v16 = stream.astype(np.int16).reshape(-1, 16).T  # [16, n/16]
        idx_streams.append(np.ascontiguousarray(np.tile(v16, (8, 1))))

    # layer-1 table precomputed on host: [TAB_ROWS, 128] bf16 rows
    # [feat(64) | el | er | 0...], pad rows have el = -1e15.
    nw = np.asarray(node_weight, np.float32)
    tab0 = np.zeros((TAB_ROWS, ROW), np.float32)
    er0 = np.zeros((N_CORES, P, N_TILES), np.float32)
    for c in range(N_CORES):
        rows = orig_of[c * SHARD: (c + 1) * SHARD]
        feat = nw[rows] @ Ws[0]                       # [SHARD, 64]
        el = feat @ als[0]
        er = feat @ ars[0]
        r0 = c * SHARD_P
        tab0[r0:r0 + SHARD, 0:D] = feat
        tab0[r0:r0 + SHARD, 64] = el
        tab0[r0:r0 + SHARD, 65] = er
        tab0[r0 + SHARD, 64:66] = PAD_EL              # pad row
        erp = np.zeros(N_TILES * P, np.float32)
        erp[:SHARD] = er
        er0[c] = erp.reshape(N_TILES, P).T
    tab0 = tab0.astype(ml_dtypes.bfloat16)

    # Wstack per layer [64, 66] = [W | W@al | W@ar], bf16
    wstk = np.concatenate(
        [np.concatenate([Ws[l], (Ws[l] @ als[l])[:, None], (Ws[l] @ ars[l])[:, None]],
                        axis=1)[None] for l in range(3)], axis=0
    ).astype(np.float32)  # [3, 64, 66]

    return dict(chunks=chunks, W_cw=W_cw, call_meta=call_meta,
                idx_streams=idx_streams, tab0=tab0, er0=er0, wstk=wstk,
                orig_of=orig_of, total_cols=total_cols)


# ---------------------------------------------------------------- device side
def _build_nc(call_meta, C16):
    nc = bacc.Bacc("TRN2", target_bir_lowering=False, debug=False,
                   num_devices=N_CORES)

    tab0_in = nc.dram_tensor("tab0", [TAB_ROWS, ROW], bf16, kind="ExternalInput")
    er_in = nc.dram_tensor("er0", [P, N_TILES], f32, kind="ExternalInput")
    idx_in = nc.dram_tensor("idx", [P, C16], i16, kind="ExternalInput")
    wstk_in = nc.dram_tensor("wstk", [3, D, 66], f32, kind="ExternalInput")
    bnp_in = nc.dram_tensor("bnp", [3, 2, D], f32, kind="ExternalInput")  # g, beta
    out_t = nc.dram_tensor("out", [SHARD, D], f32, kind="ExternalOutput")

    rg = [list(range(N_CORES))]
    nc.gpsimd.load_library(mlp_lib)

    with tile.TileContext(nc) as tc:
        with (
            tc.tile_pool(name="const", bufs=1) as constp,
            tc.tile_pool(name="gbuf", bufs=6) as gbuf,
            tc.tile_pool(name="idxb", bufs=2) as idxb,
            tc.tile_pool(name="wfp", bufs=1) as wfp,
            tc.tile_pool(name="exep", bufs=2) as exep,
            tc.tile_pool(name="redp", bufs=2) as redp,
            tc.tile_pool(name="small", bufs=3) as small,
            tc.tile_pool(name="acc", bufs=1) as accp,
            tc.tile_pool(name="stg", bufs=2) as stgp,
            tc.tile_pool(name="ps", bufs=2, space="PSUM") as ps,
            tc.tile_pool(name="pst", bufs=1, space="PSUM") as pst,
            tc.tile_pool(name="dram", bufs=1, space="DRAM") as dram,
        ):
            ident = constp.tile([P, P], f32)
            make_identity(nc, ident[:])
            ones_col = constp.tile([P, 1], f32)
            nc.vector.memset(ones_col[:], 1.0)
            ones_bf = constp.tile([P, 1], f16)
            nc.vector.memset(ones_bf[:], 1.0)
            wstk_t = constp.tile([D, 3 * 66], f32)
            nc.sync.dma_start(
                out=wstk_t[:].rearrange("k (l n) -> k l n", n=66),
                in_=wstk_in[:, :, :].rearrange("l k n -> k l n"))
            padrow = constp.tile([1, ROW], bf16)
            nc.vector.memset(padrow[:], 0.0)
            nc.vector.memset(padrow[:, 64:66], PAD_EL)

            er_sb = constp.tile([P, N_TILES], f32)
            nc.sync.dma_start(out=er_sb[:], in_=er_in[:, :])
            out_sb = accp.tile([P, N_TILES * D], f32)

            tab1 = dram.tile([TAB_ROWS, ROW], bf16, name="tab1")
            tab2 = dram.tile([TAB_ROWS, ROW], bf16, name="tab2")
            tables = [[tab0_in[w * WIN_ROWS:(w + 1) * WIN_ROWS, :] for w in range(N_WIN)],
                      [tab1[w * WIN_ROWS:(w + 1) * WIN_ROWS, :] for w in range(N_WIN)],
                      [tab2[w * WIN_ROWS:(w + 1) * WIN_ROWS, :] for w in range(N_WIN)]]
            ag_tabs = [None, tab1, tab2]
            shard_buf = dram.tile([SHARD_P, ROW], bf16)
            stats_dram_in = dram.tile([D, 2], f32)
            stats_dram_out = dram.tile([D, 2], f32)
            bcast_dram = dram.tile([3, D], f32)

            # ---------------- per-layer gather + aggregate ----------------
            statall = pst.tile([D, 8], f32, name="statall")
            for l in range(N_LAYERS):
                table = tables[l]
                stat = statall[:, 2 * l: 2 * l + 2]
                for ic, meta in enumerate(call_meta):
                    ch = meta["tiles"]
                    T = len(ch)
                    t0c = ch[0]
                    nact = len(meta["calls"])
                    it = idxb.tile([P, meta["ncols"] * P // 16], i16, tag="idx")
                    nc.sync.dma_start(
                        out=it[:],
                        in_=idx_in[:, meta["off16"]: meta["off16"] + meta["ncols"] * P // 16])
                    esum = small.tile([P, nact * T], f32, tag="esum")
                    red = redp.tile([P, nact * T * D], f16, tag="red")
                    for wi, (w, W, coloff, n_idx) in enumerate(meta["calls"]):
                        gt = gbuf.tile([P, T * W * ROW], bf16, tag="g")
                        nc.gpsimd.dma_gather(
                            out_ap=gt[:].rearrange("p (c r) -> p c r", r=ROW),
                            in_ap=table[w],
                            idxs_ap=it[:, coloff * P // 16: (coloff + T * W) * P // 16],
                            num_idxs=n_idx,
                            num_idxs_reg=n_idx,
                            elem_size=ROW,
                            single_packet=False,
                        )
                        g4 = gt[:].rearrange("p (t w r) -> p t w r", w=W, r=ROW)
                        lb = small.tile([P, T * W], f32, tag="lb")
                        nc.vector.tensor_tensor(
                            out=lb[:].rearrange("p (t w) -> p t w", w=W),
                            in0=g4[:, :, :, 64:65].rearrange("p t w o -> p t (w o)"),
                            in1=er_sb[:, t0c:t0c + T].unsqueeze(2).to_broadcast([P, T, W]),
                            op=mybir.AluOpType.add)
                        nc.scalar.activation(
                            out=lb[:], in_=lb[:],
                            func=mybir.ActivationFunctionType.Prelu,
                            alpha=NEG_SLOPE)
                        exw = small.tile([P, T * W], f32, tag="exw")
                        nc.scalar.activation(
                            out=exw[:], in_=lb[:],
                            func=mybir.ActivationFunctionType.Exp)
                        nc.vector.tensor_reduce(
                            out=esum[:, wi * T:(wi + 1) * T],
                            in_=exw[:].rearrange("p (t w) -> p t w", w=W),
                            axis=mybir.AxisListType.X, op=mybir.AluOpType.add)
                        # dense bf16 expansion of ex on ACT so the mult runs
                        # in the 2x 16-bit DVE mode (all operands 2B stride-1)
                        exe = exep.tile([P, T * W * D], f16, tag="exe")
                        nc.scalar.activation(
                            out=exe[:].rearrange("p (t w d) -> p t w d", w=W, d=D),
                            in_=exw[:].rearrange("p (t w) -> p t w", w=W)
                                .unsqueeze(3).to_broadcast([P, T, W, D]),
                            func=mybir.ActivationFunctionType.Copy)
                        wf = wfp.tile([P, T * W * D], f16, tag="wf")
                        nc.vector.tensor_tensor(
                            out=wf[:].rearrange("p (t w d) -> p t w d", w=W, d=D),
                            in0=g4[:, :, :, 0:D],
                            in1=exe[:].rearrange("p (t w d) -> p t w d", w=W, d=D),
                            op=mybir.AluOpType.mult)
                        with nc.allow_low_precision(reason="fp16 window partials"):
                            nc.vector.tensor_reduce(
                                out=red[:, wi * T * D:(wi + 1) * T * D].rearrange(
                                    "p (t d) -> p t d", d=D),
                                in_=wf[:].rearrange("p (t w d) -> p t d w", w=W, d=D),
                                axis=mybir.AxisListType.X, op=mybir.AluOpType.add)
                    rs = small.tile([P, T], f32, tag="rs")
                    if nact > 1:
                        nc.vector.tensor_reduce(
                            out=rs[:], in_=esum[:].rearrange("p (w t) -> p t w", t=T),
                            axis=mybir.AxisListType.X, op=mybir.AluOpType.add)
                    else:
                        nc.vector.tensor_copy(out=rs[:], in_=esum[:])
                    # +eps so empty/padded lanes yield 0 (not NaN) after scale
                    nc.vector.tensor_scalar(out=rs[:], in0=rs[:], scalar1=1e-20,
                                            scalar2=None, op0=mybir.AluOpType.add)
                    rinv = small.tile([P, T], f32, tag="ri")
                    nc.vector.reciprocal(out=rinv[:], in_=rs[:])
                    osum = small.tile([P, T * D], f32, tag="osum")
                    if nact > 1:
                        nc.vector.tensor_reduce(
                            out=osum[:],
                            in_=red[:].rearrange("p (w q) -> p q w", w=nact),
                            axis=mybir.AxisListType.X, op=mybir.AluOpType.add)
                    else:
                        nc.vector.tensor_copy(out=osum[:], in_=red[:])
                    sq = small.tile([P, T * D], f16, tag="sq")
                    for k, t in enumerate(ch):
                        nc.vector.tensor_scalar(
                            out=out_sb[:, t * D:(t + 1) * D],
                            in0=osum[:, k * D:(k + 1) * D],
                            scalar1=rinv[:, k:k + 1], scalar2=None,
                            op0=mybir.AluOpType.mult)
                    # incremental BN stats: square on ACT, PSUM-chained matmuls
                    nc.scalar.activation(
                        out=sq[:], in_=out_sb[:, ch[0] * D:(ch[-1] + 1) * D],
                        func=mybir.ActivationFunctionType.Square)
                    for k, t in enumerate(ch):
                        m = P if t < N_TILES - 1 else LAST_TILE_N
                        first = ic == 0 and k == 0
                        last = t == N_TILES - 1
                        nc.tensor.matmul(out=stat[:, 0:1],
                                         lhsT=out_sb[:m, t * D:(t + 1) * D],
                                         rhs=ones_col[:m, :],
                                         start=first, stop=last)
                        nc.tensor.matmul(out=stat[:, 1:2],
                                         lhsT=sq[:m, k * D:(k + 1) * D],
                                         rhs=ones_bf[:m, :],
                                         start=first, stop=last)
                stat_sb = small.tile([D, 2], f32, tag="stc")
                nc.vector.tensor_copy(out=stat_sb[:], in_=stat)
                nc.scalar.dma_start(out=stats_dram_in[:], in_=stat_sb[:])
                if not NO_COLL:
                    nc.gpsimd.collective_compute(
                        "AllReduce", mybir.AluOpType.add, replica_groups=rg,
                        ins=[stats_dram_in.opt()], outs=[stats_dram_out.opt()])
                stat_g = small.tile([D, 2], f32, tag="stg")
                nc.scalar.dma_start(
                    out=stat_g[:],
                    in_=(stats_dram_in if NO_COLL else stats_dram_out)[:])
                mu = small.tile([D, 1], f32, tag="mu")
                nc.vector.tensor_scalar(out=mu[:], in0=stat_g[:, 0:1],
                                        scalar1=1.0 / N_NODES, scalar2=None,
                                        op0=mybir.AluOpType.mult)
                musq = small.tile([D, 1], f32, tag="musq")
                nc.scalar.activation(out=musq[:], in_=mu[:],
                                     func=mybir.ActivationFunctionType.Square)
                var = small.tile([D, 1], f32, tag="var")
                nc.vector.tensor_scalar(out=var[:], in0=stat_g[:, 1:2],
                                        scalar1=1.0 / N_NODES, scalar2=BN_EPS,
                                        op0=mybir.AluOpType.mult,
                                        op1=mybir.AluOpType.add)
                nc.vector.tensor_tensor(out=var[:], in0=var[:], in1=musq[:],
                                        op=mybir.AluOpType.subtract)
                sd = small.tile([D, 1], f32, tag="sd")
                nc.scalar.activation(out=sd[:], in_=var[:],
                                     func=mybir.ActivationFunctionType.Sqrt)
                rstd = small.tile([D, 1], f32, tag="rstd")
                nc.vector.reciprocal(out=rstd[:], in_=sd[:])
                gcol = small.tile([D, 1], f32, tag="gc")
                nc.scalar.dma_start(out=gcol[:], in_=bnp_in[l, 0, :][:, None])
                bcol = small.tile([D, 1], f32, tag="bc")
                nc.scalar.dma_start(out=bcol[:], in_=bnp_in[l, 1, :][:, None])
                grs = small.tile([D, 1], f32, tag="grs")
                nc.vector.tensor_tensor(out=grs[:], in0=gcol[:], in1=rstd[:],
                                        op=mybir.AluOpType.mult)
                negmu = small.tile([D, 1], f32, tag="nmu")
                nc.vector.tensor_scalar(out=negmu[:], in0=mu[:], scalar1=-1.0,
                                        scalar2=None, op0=mybir.AluOpType.mult)

                if l < N_LAYERS - 1:
                    # transform in groups of 4 tiles: transpose, BN, ELU, project
                    for g0 in range(0, N_TILES, 4):
                        tl = list(range(g0, min(g0 + 4, N_TILES)))
                        ncol = sum(P if t < N_TILES - 1 else LAST_TILE_N for t in tl)
                        pT = ps.tile([D, 4 * P], f32, tag="pT")
                        for k, t in enumerate(tl):
                            m = P if t < N_TILES - 1 else LAST_TILE_N
                            nc.tensor.transpose(
                                out=pT[:, k * P: k * P + m],
                                in_=out_sb[:m, t * D:(t + 1) * D],
                                identity=ident[:m, :m])
                        # packed column view (partial last tile packs left)
                        z = small.tile([D, 4 * P], f32, tag="z")
                        nc.vector.tensor_scalar(
                            out=z[:, :], in0=pT[:, :],
                            scalar1=negmu[:, :], scalar2=grs[:, :],
                            op0=mybir.AluOpType.add, op1=mybir.AluOpType.mult)
                        ez = small.tile([D, 4 * P], f32, tag="ez")
                        nc.scalar.activation(out=ez[:], in_=z[:],
                                             func=mybir.ActivationFunctionType.Exp,
                                             bias=bcol[:, :])
                        nc.vector.tensor_scalar(
                            out=ez[:], in0=ez[:], scalar1=-1.0, scalar2=0.0,
                            op0=mybir.AluOpType.add, op1=mybir.AluOpType.min)
                        nc.vector.tensor_scalar(
                            out=z[:], in0=z[:], scalar1=bcol[:, :], scalar2=0.0,
                            op0=mybir.AluOpType.add, op1=mybir.AluOpType.max)
                        h2 = small.tile([D, 4 * P], f32, tag="h2")
                        nc.vector.tensor_tensor(out=h2[:], in0=z[:], in1=ez[:],
                                                op=mybir.AluOpType.add)
                        stage = stgp.tile([P, 4 * 66], bf16, tag="tfs")
                        pjt = ps.tile([P, 4 * 66], f32, tag="pj")
                        for k, t in enumerate(tl):
                            m = P if t < N_TILES - 1 else LAST_TILE_N
                            nc.tensor.matmul(
                                out=pjt[:m, k * 66:(k + 1) * 66],
                                lhsT=h2[:, k * P: k * P + m],
                                rhs=wstk_t[:, (l + 1) * 66:(l + 2) * 66],
                                start=True, stop=True)
                            nc.scalar.copy(out=stage[:m, k * 66:(k + 1) * 66],
                                           in_=pjt[:m, k * 66:(k + 1) * 66])
                            nc.vector.tensor_copy(
                                out=er_sb[:m, t:t + 1],
                                in_=pjt[:m, k * 66 + 65: k * 66 + 66])
                        if tl[-1] < N_TILES - 1:
                            nc.scalar.dma_start(
                                out=shard_buf[g0 * P: g0 * P + 4 * P, 0:66].rearrange(
                                    "(k p) n -> p k n", p=P),
                                in_=stage[:, :].rearrange("p (k n) -> p k n", n=66))
                        else:
                            for k, t in enumerate(tl):
                                m = P if t < N_TILES - 1 else LAST_TILE_N
                                nc.scalar.dma_start(
                                    out=shard_buf[t * P: t * P + m, 0:66],
                                    in_=stage[:m, k * 66:(k + 1) * 66])
                    nc.sync.dma_start(out=shard_buf[SHARD:SHARD + 1, :],
                                      in_=padrow[:, :])
                    if not NO_COLL:
                        nc.gpsimd.collective_compute(
                            "AllGather", mybir.AluOpType.bypass, replica_groups=rg,
                            ins=[shard_buf.opt()], outs=[ag_tabs[l + 1].opt()])
                else:
                    # final BN in node-major with row-broadcast params
                    nc.scalar.dma_start(out=bcast_dram[0, :], in_=negmu[:, 0])
                    nc.scalar.dma_start(out=bcast_dram[1, :], in_=grs[:, 0])
                    nc.scalar.dma_start(out=bcast_dram[2, :], in_=bcol[:, 0])
                    brow = small.tile([P, 3 * D], f32, tag="brow")
                    nc.scalar.dma_start(
                        out=brow[:],
                        in_=bcast_dram[:, :].rearrange("a b -> (a b)")[None, :]
                            .to_broadcast([P, 3 * D]))
                    GO = 8
                    for g0 in range(0, N_TILES, GO):
                        gn = min(GO, N_TILES - g0)
                        gnf = gn if g0 + gn < N_TILES else gn - 1  # full tiles
                        ob = out_sb[:, g0 * D:(g0 + gn) * D].rearrange(
                            "p (t d) -> p t d", d=D)
                        nc.vector.tensor_tensor(
                            out=ob, in0=ob,
                            in1=brow[:, 0:D].unsqueeze(1).to_broadcast([P, gn, D]),
                            op=mybir.AluOpType.add)
                        nc.vector.tensor_tensor(
                            out=ob, in0=ob,
                            in1=brow[:, D:2 * D].unsqueeze(1).to_broadcast([P, gn, D]),
                            op=mybir.AluOpType.mult)
                        nc.vector.tensor_tensor(
                            out=ob, in0=ob,
                            in1=brow[:, 2 * D:3 * D].unsqueeze(1).to_broadcast([P, gn, D]),
                            op=mybir.AluOpType.add)
                        if gnf > 0:
                            nc.sync.dma_start(
                                out=out_t[g0 * P:(g0 + gnf) * P, :].rearrange(
                                    "(t p) d -> p t d", p=P),
                                in_=out_sb[:, g0 * D:(g0 + gnf) * D].rearrange(
                                    "p (t d) -> p t d", d=D))
                        if g0 + gn == N_TILES:
                            nc.sync.dma_start(
                                out=out_t[(N_TILES - 1) * P: SHARD, :],
                                in_=out_sb[:LAST_TILE_N,
                                           (N_TILES - 1) * D: N_TILES * D])

    nc.compile()
    return nc


_CACHE = {}


def kernel(node_weight, edge_weight, src, dst,
           W1, al1, ar1, b1, g1, beta1,
           W2, al2, ar2, b2, g2, beta2,
           W3, al3, ar3, b3, g3, beta3):
    Ws = [np.asarray(W1, np.float32), np.asarray(W2, np.float32), np.asarray(W3, np.float32)]
    als = [np.asarray(al1, np.float32), np.asarray(al2, np.float32), np.asarray(al3, np.float32)]
    ars = [np.asarray(ar1, np.float32), np.asarray(ar2, np.float32), np.asarray(ar3, np.float32)]
    pre = _preprocess(node_weight, src, dst, Ws, als, ars)

    C16 = pre["idx_streams"][0].shape[1]
    key = ("nc", C16, N_LAYERS, NO_COLL,
           tuple(tuple(w) for w in pre["W_cw"]),
           tuple(len(m["tiles"]) for m in pre["call_meta"]))
    if key not in _CACHE:
        _CACHE[key] = _build_nc(pre["call_meta"], C16)
    nc = _CACHE[key]

    # bnp: [3, 2, 64] = (g, beta) per layer; the GAT bias b cancels in BN.
    bnp = np.stack([
        np.stack([np.asarray(g, np.float32), np.asarray(be, np.float32)])
        for g, be in ((g1, beta1), (g2, beta2), (g3, beta3))
    ])

    in_maps = []
    for c in range(N_CORES):
        in_maps.append({
            "tab0": pre["tab0"],
            "er0": pre["er0"][c],
            "idx": pre["idx_streams"][c],
            "wstk": pre["wstk"],
            "bnp": bnp,
        })
    res = bass_utils.run_bass_kernel_spmd(nc, in_maps, core_ids=list(range(N_CORES)))

    out = np.empty((N_NODES, D), np.float32)
    for c in range(N_CORES):
        rows = pre["orig_of"][c * SHARD: (c + 1) * SHARD]
        out[rows] = res.results[c]["out"]
    return out


# revision 64
# speedup vs baseline: 1.3458x; 1.0058x over previous
"""3-layer GAT on 8 TRN2 NeuronCores via Bass/Tile.

Architecture (v2):
- Nodes dst-sharded 12500/core, re-sorted by (argmax window, -max window count).
- Per-layer node table in each core's DRAM: [100008, 128] bf16 rows
  [feat(64) | el | er | pad], 4 windows of 25002 rows (row 12500 of each
  window = pad row with el = -1e15 so padded slots contribute exp(...)=0).
  The layer-1 table (a pure linear fold of the inputs) is precomputed on the
  host and passed as an input, so layer-1 gathers start immediately.
- Edge gather via InstDMAGatherAnt (int16 idx) per (chunk, window) with
  chunk-uniform width W_cw: grid [P, T, W, 128] bf16. Chunk boundaries chosen
  by DP to minimize padded slots + fixed per-chunk overhead.
- Per (chunk, window): DVE logit-add (+er broadcast), ACT Prelu, ACT Exp,
  DVE exsum-reduce; ACT expands ex to a dense fp16 [P,T,W,64] so the weighted
  mult runs in the 2x 16-bit DVE mode; DVE reduce to fp16 window partials;
  per-chunk window combine + 1/(sum+eps) scale. The GAT bias b provably
  cancels through BatchNorm and is dropped; eps makes isolated nodes exact 0.
- BN stats incrementally per chunk (ACT square + PSUM-chained PE ones-matmuls)
  + AllReduce; inter-layer transform in groups of 4 tiles (PE transpose, fused
  BN, ELU with exp-bias fold, projection); AllGather of projected shard
  tables. Engine queues: Pool = gathers only, SP = idx/outputs, ACT = small
  DMAs + activations, so no head-of-line blocking on the gather path.
"""
import sys
sys.path.insert(0, "/opt/trn_rl_repo")
import os
import numpy as np
import ml_dtypes

import concourse.bass as bass
import concourse.bacc as bacc
import concourse.tile as tile
import concourse.mybir as mybir
from concourse import bass_utils
from concourse.library_config import mlp as mlp_lib
from concourse.masks import make_identity

N_NODES = 100000
N_EDGES = 1600000
D = 64
N_CORES = 8
SHARD = 12500
SHARD_P = SHARD + 1          # + pad row
N_WIN = 4
WIN_ROWS = 2 * SHARD_P       # 25002 rows per window
TAB_ROWS = N_CORES * SHARD_P # 100008
ROW = 128                    # bf16 elems per table row (256B)
NEG_SLOPE = 0.2
BN_EPS = 1e-5
P = 128
N_TILES = (SHARD + P - 1) // P          # 98 (last tile 84 nodes)
LAST_TILE_N = SHARD - (N_TILES - 1) * P  # 84
PAD_EL = -1e15
WIN_PAD = 12500              # window-relative pad row
N_LAYERS = int(os.environ.get("GAT_LAYERS", "3"))
NO_COLL = os.environ.get("GAT_NO_COLL", "0") == "1"
# DP chunking knobs
CHUNK_FIXED_SLOTS = 2000     # fixed per-chunk cost in slot-equivalents
CHUNK_MAX_TILES = 14
CHUNK_MAX_COLS = 64          # T * max_w W_cw  (bounds SBUF grid size)

f32 = mybir.dt.float32
bf16 = mybir.dt.bfloat16
i16 = mybir.dt.int16
f16 = mybir.dt.float16


# ---------------------------------------------------------------- host side
def _chunk_dp(W_t):
    """Pick chunk boundaries minimizing sum(T*sum_w max W) * P + fixed."""
    n = W_t.shape[0]
    INF = float("inf")
    dp = [INF] * (n + 1)
    prv = [0] * (n + 1)
    dp[0] = 0.0
    for j in range(1, n + 1):
        for i in range(max(0, j - CHUNK_MAX_TILES), j):
            T = j - i
            Wm = W_t[i:j].max(axis=0)
            if T * Wm.max() > CHUNK_MAX_COLS:
                continue
            c = dp[i] + T * Wm.sum() * P + CHUNK_FIXED_SLOTS
            if c < dp[j]:
                dp[j] = c
                prv[j] = i
    bounds = []
    j = n
    while j > 0:
        i = prv[j]
        bounds.append((i, j))
        j = i
    bounds.reverse()
    return [list(range(i, j)) for i, j in bounds]


def _preprocess(node_weight, src, dst, Ws, als, ars):
    src = np.asarray(src).astype(np.int64)
    dst = np.asarray(dst).astype(np.int64)

    # per-(node, window) incoming-edge counts; window of a src node depends
    # only on its shard (fixed), not the within-shard order.
    src_win = (src // SHARD) // 2
    cnt_w = np.zeros((N_NODES, N_WIN), np.int64)
    np.add.at(cnt_w, (dst, src_win), 1)

    newid = np.empty(N_NODES, np.int64)
    orig_of = np.empty(N_NODES, np.int64)  # compact (core*SHARD+rank) -> orig
    for c in range(N_CORES):
        orig = np.arange(c * SHARD, (c + 1) * SHARD)
        cw = cnt_w[orig]
        order = orig[np.lexsort((cw.argmax(1), -cw.max(1)))]
        newid[order] = c * SHARD_P + np.arange(SHARD)
        orig_of[c * SHARD: (c + 1) * SHARD] = order

    src_n = newid[src]
    dst_core = dst // SHARD
    dst_loc = newid[dst] % SHARD_P       # local rank within shard [0, 12500)
    win_of_src = src_n // WIN_ROWS

    # per-core grouped edges + per-tile window maxima (max over cores)
    per_core = []
    W_t = np.zeros((N_TILES, N_WIN), np.int64)
    for c in range(N_CORES):
        m = dst_core == c
        s_c, d_c, w_c = src_n[m], dst_loc[m], win_of_src[m]
        o = np.lexsort((s_c, w_c, d_c))
        s_c, d_c, w_c = s_c[o], d_c[o], w_c[o]
        cnt = np.zeros((SHARD, N_WIN), np.int64)
        np.add.at(cnt, (d_c, w_c), 1)
        per_core.append((s_c, d_c, w_c, cnt))
        cm = cnt.reshape(-1)[: (N_TILES - 1) * P * N_WIN].reshape(N_TILES - 1, P, N_WIN).max(axis=1)
        W_t[: N_TILES - 1] = np.maximum(W_t[: N_TILES - 1], cm)
        W_t[N_TILES - 1] = np.maximum(
            W_t[N_TILES - 1], cnt[(N_TILES - 1) * P:].max(axis=0))

    chunks = _chunk_dp(W_t)
    # per (chunk, window) uniform width + call metadata (uniform across cores)
    W_cw = [[int(W_t[ch].max(axis=0)[w]) for w in range(N_WIN)] for ch in chunks]
    call_meta = []  # list per chunk: dict(cols0, len16, calls=[(w, W, coloff, n_idx)])
    off16 = 0
    col = 0
    for ci, ch in enumerate(chunks):
        T = len(ch)
        calls = []
        cols0 = col
        for w in range(N_WIN):
            W = W_cw[ci][w]
            if W == 0:
                continue
            calls.append((w, W, col - cols0, T * W * P))
            col += T * W
        call_meta.append(dict(cols0=cols0, ncols=col - cols0, off16=off16,
                              calls=calls, tiles=ch))
        off16 += (col - cols0) * P // 16
    total_cols = col

    # per-core idx streams (vectorized fill)
    tile_of = np.zeros(N_TILES, np.int64)
    tloc_of = np.zeros(N_TILES, np.int64)
    chunk_of = np.zeros(N_TILES, np.int64)
    for ci, ch in enumerate(chunks):
        for k, t in enumerate(ch):
            chunk_of[t] = ci
            tloc_of[t] = k
    # column offset of (chunk, window) in global stream
    colbase = np.zeros((len(chunks), N_WIN), np.int64)
    for ci, meta in enumerate(call_meta):
        for (w, W, coloff, _n) in meta["calls"]:
            colbase[ci, w] = meta["cols0"] + coloff
    Wmat = np.array([[W_cw[ci][w] for w in range(N_WIN)]
                     for ci in range(len(chunks))], np.int64)

    idx_streams = []
    for c in range(N_CORES):
        s_c, d_c, w_c, cnt = per_core[c]
        key = d_c * N_WIN + w_c
        run_start = np.zeros(SHARD * N_WIN + 1, np.int64)
        np.add.at(run_start, key + 1, 1)
        run_start = np.cumsum(run_start)
        rank = np.arange(len(s_c)) - run_start[key]
        t = d_c // P
        p = d_c % P
        ci = chunk_of[t]
        colg = colbase[ci, w_c] + tloc_of[t] * Wmat[ci, w_c] + rank
        stream = np.full(total_cols * P, WIN_PAD, np.int64)
        stream[colg * P + p] = s_c - w_c * WIN_ROWS
        v16 = stream.astype(np.int16).reshape(-1, 16).T  # [16, n/16]
        idx_streams.append(np.ascontiguousarray(np.tile(v16, (8, 1))))

    # layer-1 table precomputed on host: [TAB_ROWS, 128] bf16 rows
    # [feat(64) | el | er | 0...], pad rows have el = -1e15.
    nw = np.asarray(node_weight, np.float32)
    tab0 = np.zeros((TAB_ROWS, ROW), np.float32)
    er0 = np.zeros((N_CORES, P, N_TILES), np.float32)
    for c in range(N_CORES):
        rows = orig_of[c * SHARD: (c + 1) * SHARD]
        feat = nw[rows] @ Ws[0]                       # [SHARD, 64]
        el = feat @ als[0]
        er = feat @ ars[0]
        r0 = c * SHARD_P
        tab0[r0:r0 + SHARD, 0:D] = feat
        tab0[r0:r0 + SHARD, 64] = el
        tab0[r0:r0 + SHARD, 65] = er
        tab0[r0 + SHARD, 64:66] = PAD_EL              # pad row
        erp = np.zeros(N_TILES * P, np.float32)
        erp[:SHARD] = er
        er0[c] = erp.reshape(N_TILES, P).T
    tab0 = tab0.astype(ml_dtypes.bfloat16)

    # Wstack per layer [64, 66] = [W | W@al | W@ar], bf16
    wstk = np.concatenate(
        [np.concatenate([Ws[l], (Ws[l] @ als[l])[:, None], (Ws[l] @ ars[l])[:, None]],
                        axis=1)[None] for l in range(3)], axis=0
    ).astype(np.float32)  # [3, 64, 66]

    return dict(chunks=chunks, W_cw=W_cw, call_meta=call_meta,
                idx_streams=idx_streams, tab0=tab0, er0=er0, wstk=wstk,
                orig_of=orig_of, total_cols=total_cols)


# ---------------------------------------------------------------- device side
def _build_nc(call_meta, C16):
    nc = bacc.Bacc("TRN2", target_bir_lowering=False, debug=False,
                   num_devices=N_CORES)

    tab0_in = nc.dram_tensor("tab0", [TAB_ROWS, ROW], bf16, kind="ExternalInput")
    er_in = nc.dram_tensor("er0", [P, N_TILES], f32, kind="ExternalInput")
    idx_in = nc.dram_tensor("idx", [P, C16], i16, kind="ExternalInput")
    wstk_in = nc.dram_tensor("wstk", [3, D, 66], f32, kind="ExternalInput")
    bnp_in = nc.dram_tensor("bnp", [3, 2, D], f32, kind="ExternalInput")  # g, beta
    out_t = nc.dram_tensor("out", [SHARD, D], f32, kind="ExternalOutput")

    rg = [list(range(N_CORES))]
    nc.gpsimd.load_library(mlp_lib)

    with tile.TileContext(nc) as tc:
        with (
            tc.tile_pool(name="const", bufs=1) as constp,
            tc.tile_pool(name="gbuf", bufs=6) as gbuf,
            tc.tile_pool(name="idxb", bufs=2) as idxb,
            tc.tile_pool(name="wfp", bufs=1) as wfp,
            tc.tile_pool(name="redp", bufs=2) as redp,
            tc.tile_pool(name="small", bufs=3) as small,
            tc.tile_pool(name="acc", bufs=1) as accp,
            tc.tile_pool(name="stg", bufs=2) as stgp,
            tc.tile_pool(name="ps", bufs=2, space="PSUM") as ps,
            tc.tile_pool(name="pst", bufs=1, space="PSUM") as pst,
            tc.tile_pool(name="dram", bufs=1, space="DRAM") as dram,
        ):
            ident = constp.tile([P, P], f32)
            make_identity(nc, ident[:])
            ones_col = constp.tile([P, 1], f32)
            nc.vector.memset(ones_col[:], 1.0)
            ones_bf = constp.tile([P, 1], f16)
            nc.vector.memset(ones_bf[:], 1.0)
            wstk_t = constp.tile([D, 3 * 66], f32)
            nc.sync.dma_start(
                out=wstk_t[:].rearrange("k (l n) -> k l n", n=66),
                in_=wstk_in[:, :, :].rearrange("l k n -> k l n"))
            padrow = constp.tile([1, ROW], bf16)
            nc.vector.memset(padrow[:], 0.0)
            nc.vector.memset(padrow[:, 64:66], PAD_EL)

            er_sb = constp.tile([P, N_TILES], f32)
            nc.sync.dma_start(out=er_sb[:], in_=er_in[:, :])
            out_sb = accp.tile([P, N_TILES * D], f32)

            tab1 = dram.tile([TAB_ROWS, ROW], bf16, name="tab1")
            tab2 = dram.tile([TAB_ROWS, ROW], bf16, name="tab2")
            tables = [[tab0_in[w * WIN_ROWS:(w + 1) * WIN_ROWS, :] for w in range(N_WIN)],
                      [tab1[w * WIN_ROWS:(w + 1) * WIN_ROWS, :] for w in range(N_WIN)],
                      [tab2[w * WIN_ROWS:(w + 1) * WIN_ROWS, :] for w in range(N_WIN)]]
            ag_tabs = [None, tab1, tab2]
            shard_buf = dram.tile([SHARD_P, ROW], bf16)
            stats_dram_in = dram.tile([D, 2], f32)
            stats_dram_out = dram.tile([D, 2], f32)
            bcast_dram = dram.tile([3, D], f32)

            # ---------------- per-layer gather + aggregate ----------------
            statall = pst.tile([D, 8], f32, name="statall")
            for l in range(N_LAYERS):
                table = tables[l]
                stat = statall[:, 2 * l: 2 * l + 2]
                for ic, meta in enumerate(call_meta):
                    ch = meta["tiles"]
                    T = len(ch)
                    t0c = ch[0]
                    nact = len(meta["calls"])
                    it = idxb.tile([P, meta["ncols"] * P // 16], i16, tag="idx")
                    nc.sync.dma_start(
                        out=it[:],
                        in_=idx_in[:, meta["off16"]: meta["off16"] + meta["ncols"] * P // 16])
                    esum = small.tile([P, nact * T], f32, tag="esum")
                    red = redp.tile([P, nact * T * D], f16, tag="red")
                    for wi, (w, W, coloff, n_idx) in enumerate(meta["calls"]):
                        gt = gbuf.tile([P, T * W * ROW], bf16, tag="g")
                        nc.gpsimd.dma_gather(
                            out_ap=gt[:].rearrange("p (c r) -> p c r", r=ROW),
                            in_ap=table[w],
                            idxs_ap=it[:, coloff * P // 16: (coloff + T * W) * P // 16],
                            num_idxs=n_idx,
                            num_idxs_reg=n_idx,
                            elem_size=ROW,
                            single_packet=False,
                        )
                        g4 = gt[:].rearrange("p (t w r) -> p t w r", w=W, r=ROW)
                        lb = small.tile([P, T * W], f32, tag="lb")
                        nc.vector.tensor_tensor(
                            out=lb[:].rearrange("p (t w) -> p t w", w=W),
                            in0=g4[:, :, :, 64:65].rearrange("p t w o -> p t (w o)"),
                            in1=er_sb[:, t0c:t0c + T].unsqueeze(2).to_broadcast([P, T, W]),
                            op=mybir.AluOpType.add)
                        nc.scalar.activation(
                            out=lb[:], in_=lb[:],
                            func=mybir.ActivationFunctionType.Prelu,
                            alpha=NEG_SLOPE)
                        exw = small.tile([P, T * W], f32, tag="exw")
                        nc.scalar.activation(
                            out=exw[:], in_=lb[:],
                            func=mybir.ActivationFunctionType.Exp)
                        nc.vector.tensor_reduce(
                            out=esum[:, wi * T:(wi + 1) * T],
                            in_=exw[:].rearrange("p (t w) -> p t w", w=W),
                            axis=mybir.AxisListType.X, op=mybir.AluOpType.add)
                        wf = wfp.tile([P, T * W * D], f16, tag="wf")
                        nc.vector.tensor_tensor(
                            out=wf[:].rearrange("p (t w d) -> p t w d", w=W, d=D),
                            in0=g4[:, :, :, 0:D],
                            in1=exw[:].rearrange("p (t w) -> p t w", w=W)
                                .unsqueeze(3).to_broadcast([P, T, W, D]),
                            op=mybir.AluOpType.mult)
                        with nc.allow_low_precision(reason="fp16 window partials"):
                            nc.vector.tensor_reduce(
                                out=red[:, wi * T * D:(wi + 1) * T * D].rearrange(
                                    "p (t d) -> p t d", d=D),
                                in_=wf[:].rearrange("p (t w d) -> p t d w", w=W, d=D),
                                axis=mybir.AxisListType.X, op=mybir.AluOpType.add)
                    rs = small.tile([P, T], f32, tag="rs")
                    if nact > 1:
                        nc.vector.tensor_reduce(
                            out=rs[:], in_=esum[:].rearrange("p (w t) -> p t w", t=T),
                            axis=mybir.AxisListType.X, op=mybir.AluOpType.add)
                    else:
                        nc.vector.tensor_copy(out=rs[:], in_=esum[:])
                    # +eps so empty/padded lanes yield 0 (not NaN) after scale
                    nc.vector.tensor_scalar(out=rs[:], in0=rs[:], scalar1=1e-20,
                                            scalar2=None, op0=mybir.AluOpType.add)
                    rinv = small.tile([P, T], f32, tag="ri")
                    nc.vector.reciprocal(out=rinv[:], in_=rs[:])
                    osum = small.tile([P, T * D], f32, tag="osum")
                    if nact > 1:
                        nc.vector.tensor_reduce(
                            out=osum[:],
                            in_=red[:].rearrange("p (w q) -> p q w", w=nact),
                            axis=mybir.AxisListType.X, op=mybir.AluOpType.add)
                    else:
                        nc.vector.tensor_copy(out=osum[:], in_=red[:])
                    sq = small.tile([P, T * D], f16, tag="sq")
                    for k, t in enumerate(ch):
                        nc.vector.tensor_scalar(
                            out=out_sb[:, t * D:(t + 1) * D],
                            in0=osum[:, k * D:(k + 1) * D],
                            scalar1=rinv[:, k:k + 1], scalar2=None,
                            op0=mybir.AluOpType.mult)
                    # incremental BN stats: square on ACT, PSUM-chained matmuls
                    nc.scalar.activation(
                        out=sq[:], in_=out_sb[:, ch[0] * D:(ch[-1] + 1) * D],
                        func=mybir.ActivationFunctionType.Square)
                    for k, t in enumerate(ch):
                        m = P if t < N_TILES - 1 else LAST_TILE_N
                        first = ic == 0 and k == 0
                        last = t == N_TILES - 1
                        nc.tensor.matmul(out=stat[:, 0:1],
                                         lhsT=out_sb[:m, t * D:(t + 1) * D],
                                         rhs=ones_col[:m, :],
                                         start=first, stop=last)
                        nc.tensor.matmul(out=stat[:, 1:2],
                                         lhsT=sq[:m, k * D:(k + 1) * D],
                                         rhs=ones_bf[:m, :],
                                         start=first, stop=last)
                stat_sb = small.tile([D, 2], f32, tag="stc")
                nc.vector.tensor_copy(out=stat_sb[:], in_=stat)
                nc.scalar.dma_start(out=stats_dram_in[:], in_=stat_sb[:])
                if not NO_COLL:
                    nc.gpsimd.collective_compute(
                        "AllReduce", mybir.AluOpType.add, replica_groups=rg,
                        ins=[stats_dram_in.opt()], outs=[stats_dram_out.opt()])
                stat_g = small.tile([D, 2], f32, tag="stg")
                nc.scalar.dma_start(
                    out=stat_g[:],
                    in_=(stats_dram_in if NO_COLL else stats_dram_out)[:])
                mu = small.tile([D, 1], f32, tag="mu")
                nc.vector.tensor_scalar(out=mu[:], in0=stat_g[:, 0:1],
                                        scalar1=1.0 / N_NODES, scalar2=None,
                                        op0=mybir.AluOpType.mult)
                musq = small.tile([D, 1], f32, tag="musq")
                nc.scalar.activation(out=musq[:], in_=mu[:],
                                     func=mybir.ActivationFunctionType.Square)
                var = small.tile([D, 1], f32, tag="var")
                nc.vector.tensor_scalar(out=var[:], in0=stat_g[:, 1:2],
                                        scalar1=1.0 / N_NODES, scalar2=BN_EPS,
                                        op0=mybir.AluOpType.mult,
                                        op1=mybir.AluOpType.add)
                nc.vector.tensor_tensor(out=var[:], in0=var[:], in1=musq[:],
                                        op=mybir.AluOpType.subtract)
                sd = small.tile([D, 1], f32, tag="sd")
                nc.scalar.activation(out=sd[:], in_=var[:],
                                     func=mybir.ActivationFunctionType.Sqrt)
                rstd = small.tile([D, 1], f32, tag="rstd")
                nc.vector.reciprocal(out=rstd[:], in_=sd[:])
                gcol = small.tile([D, 1], f32, tag="gc")
                nc.scalar.dma_start(out=gcol[:], in_=bnp_in[l, 0, :][:, None])
                bcol = small.tile([D, 1], f32, tag="bc")
                nc.scalar.dma_start(out=bcol[:], in_=bnp_in[l, 1, :][:, None])
                grs = small.tile([D, 1], f32, tag="grs")
                nc.vector.tensor_tensor(out=grs[:], in0=gcol[:], in1=rstd[:],
                                        op=mybir.AluOpType.mult)
                negmu = small.tile([D, 1], f32, tag="nmu")
                nc.vector.tensor_scalar(out=negmu[:], in0=mu[:], scalar1=-1.0,
                                        scalar2=None, op0=mybir.AluOpType.mult)

                if l < N_LAYERS - 1:
                    # transform in groups of 4 tiles: transpose, BN, ELU, project
                    for g0 in range(0, N_TILES, 4):
                        tl = list(range(g0, min(g0 + 4, N_TILES)))
                        ncol = sum(P if t < N_TILES - 1 else LAST_TILE_N for t in tl)
                        pT = ps.tile([D, 4 * P], f32, tag="pT")
                        for k, t in enumerate(tl):
                            m = P if t < N_TILES - 1 else LAST_TILE_N
                            nc.tensor.transpose(
                                out=pT[:, k * P: k * P + m],
                                in_=out_sb[:m, t * D:(t + 1) * D],
                                identity=ident[:m, :m])
                        # packed column view (partial last tile packs left)
                        z = small.tile([D, 4 * P], f32, tag="z")
                        nc.vector.tensor_scalar(
                            out=z[:, :], in0=pT[:, :],
                            scalar1=negmu[:, :], scalar2=grs[:, :],
                            op0=mybir.AluOpType.add, op1=mybir.AluOpType.mult)
                        ez = small.tile([D, 4 * P], f32, tag="ez")
                        nc.scalar.activation(out=ez[:], in_=z[:],
                                             func=mybir.ActivationFunctionType.Exp,
                                             bias=bcol[:, :])
                        nc.vector.tensor_scalar(
                            out=ez[:], in0=ez[:], scalar1=-1.0, scalar2=0.0,
                            op0=mybir.AluOpType.add, op1=mybir.AluOpType.min)
                        nc.vector.tensor_scalar(
                            out=z[:], in0=z[:], scalar1=bcol[:, :], scalar2=0.0,
                            op0=mybir.AluOpType.add, op1=mybir.AluOpType.max)
                        h2 = small.tile([D, 4 * P], f32, tag="h2")
                        nc.vector.tensor_tensor(out=h2[:], in0=z[:], in1=ez[:],
                                                op=mybir.AluOpType.add)
                        stage = stgp.tile([P, 4 * 66], bf16, tag="tfs")
                        pjt = ps.tile([P, 4 * 66], f32, tag="pj")
                        for k, t in enumerate(tl):
                            m = P if t < N_TILES - 1 else LAST_TILE_N
                            nc.tensor.matmul(
                                out=pjt[:m, k * 66:(k + 1) * 66],
                                lhsT=h2[:, k * P: k * P + m],
                                rhs=wstk_t[:, (l + 1) * 66:(l + 2) * 66],
                                start=True, stop=True)
                            nc.scalar.copy(out=stage[:m, k * 66:(k + 1) * 66],
                                           in_=pjt[:m, k * 66:(k + 1) * 66])
                            nc.vector.tensor_copy(
                                out=er_sb[:m, t:t + 1],
                                in_=pjt[:m, k * 66 + 65: k * 66 + 66])
                        if tl[-1] < N_TILES - 1:
                            nc.scalar.dma_start(
                                out=shard_buf[g0 * P: g0 * P + 4 * P, 0:66].rearrange(
                                    "(k p) n -> p k n", p=P),
                                in_=stage[:, :].rearrange("p (k n) -> p k n", n=66))
                        else:
                            for k, t in enumerate(tl):
                                m = P if t < N_TILES - 1 else LAST_TILE_N
                                nc.scalar.dma_start(
                                    out=shard_buf[t * P: t * P + m, 0:66],
                                    in_=stage[:m, k * 66:(k + 1) * 66])
                    nc.sync.dma_start(out=shard_buf[SHARD:SHARD + 1, :],
                                      in_=padrow[:, :])
                    if not NO_COLL:
                        nc.gpsimd.collective_compute(
                            "AllGather", mybir.AluOpType.bypass, replica_groups=rg,
                            ins=[shard_buf.opt()], outs=[ag_tabs[l + 1].opt()])
                else:
                    # final BN in node-major with row-broadcast params
                    nc.scalar.dma_start(out=bcast_dram[0, :], in_=negmu[:, 0])
                    nc.scalar.dma_start(out=bcast_dram[1, :], in_=grs[:, 0])
                    nc.scalar.dma_start(out=bcast_dram[2, :], in_=bcol[:, 0])
                    brow = small.tile([P, 3 * D], f32, tag="brow")
                    nc.scalar.dma_start(
                        out=brow[:],
                        in_=bcast_dram[:, :].rearrange("a b -> (a b)")[None, :]
                            .to_broadcast([P, 3 * D]))
                    GO = 8
                    for g0 in range(0, N_TILES, GO):
                        gn = min(GO, N_TILES - g0)
                        gnf = gn if g0 + gn < N_TILES else gn - 1  # full tiles
                        ob = out_sb[:, g0 * D:(g0 + gn) * D].rearrange(
                            "p (t d) -> p t d", d=D)
                        ve = nc.vector if (g0 // GO) % 2 == 0 else nc.gpsimd
                        ve.tensor_tensor(
                            out=ob, in0=ob,
                            in1=brow[:, 0:D].unsqueeze(1).to_broadcast([P, gn, D]),
                            op=mybir.AluOpType.add)
                        ve.tensor_tensor(
                            out=ob, in0=ob,
                            in1=brow[:, D:2 * D].unsqueeze(1).to_broadcast([P, gn, D]),
                            op=mybir.AluOpType.mult)
                        ve.tensor_tensor(
                            out=ob, in0=ob,
                            in1=brow[:, 2 * D:3 * D].unsqueeze(1).to_broadcast([P, gn, D]),
                            op=mybir.AluOpType.add)
                        if gnf > 0:
                            nc.sync.dma_start(
                                out=out_t[g0 * P:(g0 + gnf) * P, :].rearrange(
                                    "(t p) d -> p t d", p=P),
                                in_=out_sb[:, g0 * D:(g0 + gnf) * D].rearrange(
                                    "p (t d) -> p t d", d=D))
                        if g0 + gn == N_TILES:
                            nc.sync.dma_start(
                                out=out_t[(N_TILES - 1) * P: SHARD, :],
                                in_=out_sb[:LAST_TILE_N,
                                           (N_TILES - 1) * D: N_TILES * D])

    nc.compile()
    return nc


_CACHE = {}


def kernel(node_weight, edge_weight, src, dst,
           W1, al1, ar1, b1, g1, beta1,
           W2, al2, ar2, b2, g2, beta2,
           W3, al3, ar3, b3, g3, beta3):
    Ws = [np.asarray(W1, np.float32), np.asarray(W2, np.float32), np.asarray(W3, np.float32)]
    als = [np.asarray(al1, np.float32), np.asarray(al2, np.float32), np.asarray(al3, np.float32)]
    ars = [np.asarray(ar1, np.float32), np.asarray(ar2, np.float32), np.asarray(ar3, np.float32)]
    pre = _preprocess(node_weight, src, dst, Ws, als, ars)

    C16 = pre["idx_streams"][0].shape[1]
    key = ("nc", C16, N_LAYERS, NO_COLL,
           tuple(tuple(w) for w in pre["W_cw"]),
           tuple(len(m["tiles"]) for m in pre["call_meta"]))
    if key not in _CACHE:
        _CACHE[key] = _build_nc(pre["call_meta"], C16)
    nc = _CACHE[key]

    # bnp: [3, 2, 64] = (g, beta) per layer; the GAT bias b cancels in BN.
    bnp = np.stack([
        np.stack([np.asarray(g, np.float32), np.asarray(be, np.float32)])
        for g, be in ((g1, beta1), (g2, beta2), (g3, beta3))
    ])

    in_maps = []
    for c in range(N_CORES):
        in_maps.append({
            "tab0": pre["tab0"],
            "er0": pre["er0"][c],
            "idx": pre["idx_streams"][c],
            "wstk": pre["wstk"],
            "bnp": bnp,
        })
    res = bass_utils.run_bass_kernel_spmd(nc, in_maps, core_ids=list(range(N_CORES)))

    out = np.empty((N_NODES, D), np.float32)
    for c in range(N_CORES):
        rows = pre["orig_of"][c * SHARD: (c + 1) * SHARD]
        out[rows] = res.results[c]["out"]
    return out


# revision 72
# speedup vs baseline: 1.3718x; 1.0193x over previous
"""3-layer GAT on 8 TRN2 NeuronCores via Bass/Tile.

Architecture (v2):
- Nodes dst-sharded 12500/core, re-sorted by (argmax window, -max window count).
- Per-layer node table in each core's DRAM: [100008, 128] bf16 rows
  [feat(64) | el | er | pad], 4 windows of 25002 rows (row 12500 of each
  window = pad row with el = -1e15 so padded slots contribute exp(...)=0).
  The layer-1 table (a pure linear fold of the inputs) is precomputed on the
  host and passed as an input, so layer-1 gathers start immediately.
- Edge gather via InstDMAGatherAnt (int16 idx) per (chunk, window) with
  chunk-uniform width W_cw: grid [P, T, W, 128] bf16. Chunk boundaries chosen
  by DP to minimize padded slots + fixed per-chunk overhead.
- Per (chunk, window): DVE logit-add (+er broadcast), ACT Prelu, ACT Exp,
  DVE exsum-reduce, DVE broadcast-mult to fp16 products, DVE reduce to fp16
  window partials; per-chunk window combine + 1/(sum+eps) scale. The GAT
  bias b provably cancels through BatchNorm and is dropped; eps makes
  isolated nodes exact 0 (matching the reference for isolated nodes).
- BN stats incrementally per chunk (ACT square + PSUM-chained PE ones-matmuls)
  + AllReduce; inter-layer transform in groups of 4 tiles (PE transpose, fused
  BN, ELU with exp-bias fold, projection); AllGather of projected shard
  tables. Engine queues: Pool = gathers only, SP = idx/outputs, ACT = small
  DMAs + activations, so no head-of-line blocking on the gather path.
"""
import sys
sys.path.insert(0, "/opt/trn_rl_repo")
import os
import numpy as np
import ml_dtypes

import concourse.bass as bass
import concourse.bacc as bacc
import concourse.tile as tile
import concourse.mybir as mybir
from concourse import bass_utils
from concourse.library_config import mlp as mlp_lib
from concourse.masks import make_identity

N_NODES = 100000
N_EDGES = 1600000
D = 64
N_CORES = 8
SHARD = 12500
SHARD_P = SHARD + 1          # + pad row
N_WIN = 4
WIN_ROWS = 2 * SHARD_P       # 25002 rows per window
TAB_ROWS = N_CORES * SHARD_P # 100008
ROW = 128                    # bf16 elems per table row (256B)
NEG_SLOPE = 0.2
BN_EPS = 1e-5
P = 128
N_TILES = (SHARD + P - 1) // P          # 98 (last tile 84 nodes)
LAST_TILE_N = SHARD - (N_TILES - 1) * P  # 84
PAD_EL = -1e15
WIN_PAD = 12500              # window-relative pad row
N_LAYERS = int(os.environ.get("GAT_LAYERS", "3"))
NO_COLL = os.environ.get("GAT_NO_COLL", "0") == "1"
# DP chunking knobs
CHUNK_FIXED_SLOTS = 1000     # fixed per-chunk cost in slot-equivalents
CHUNK_MAX_TILES = 16
CHUNK_MAX_COLS = 80          # T * max_w W_cw  (bounds SBUF grid size)

f32 = mybir.dt.float32
bf16 = mybir.dt.bfloat16
i16 = mybir.dt.int16
f16 = mybir.dt.float16


# ---------------------------------------------------------------- host side
def _chunk_dp(W_t):
    """Pick chunk boundaries minimizing sum(T*sum_w max W) * P + fixed."""
    n = W_t.shape[0]
    INF = float("inf")
    dp = [INF] * (n + 1)
    prv = [0] * (n + 1)
    dp[0] = 0.0
    for j in range(1, n + 1):
        for i in range(max(0, j - CHUNK_MAX_TILES), j):
            T = j - i
            Wm = W_t[i:j].max(axis=0)
            if T * Wm.max() > CHUNK_MAX_COLS:
                continue
            c = dp[i] + T * Wm.sum() * P + CHUNK_FIXED_SLOTS
            if c < dp[j]:
                dp[j] = c
                prv[j] = i
    bounds = []
    j = n
    while j > 0:
        i = prv[j]
        bounds.append((i, j))
        j = i
    bounds.reverse()
    return [list(range(i, j)) for i, j in bounds]


def _preprocess(node_weight, src, dst, Ws, als, ars):
    src = np.asarray(src).astype(np.int64)
    dst = np.asarray(dst).astype(np.int64)

    # per-(node, window) incoming-edge counts; window of a src node depends
    # only on its shard (fixed), not the within-shard order.
    src_win = (src // SHARD) // 2
    cnt_w = np.zeros((N_NODES, N_WIN), np.int64)
    np.add.at(cnt_w, (dst, src_win), 1)

    newid = np.empty(N_NODES, np.int64)
    orig_of = np.empty(N_NODES, np.int64)  # compact (core*SHARD+rank) -> orig
    for c in range(N_CORES):
        orig = np.arange(c * SHARD, (c + 1) * SHARD)
        cw = cnt_w[orig]
        order = orig[np.lexsort((cw.argmax(1), -cw.max(1)))]
        newid[order] = c * SHARD_P + np.arange(SHARD)
        orig_of[c * SHARD: (c + 1) * SHARD] = order

    src_n = newid[src]
    dst_core = dst // SHARD
    dst_loc = newid[dst] % SHARD_P       # local rank within shard [0, 12500)
    win_of_src = src_n // WIN_ROWS

    # per-core grouped edges + per-tile window maxima (max over cores)
    per_core = []
    W_t = np.zeros((N_TILES, N_WIN), np.int64)
    for c in range(N_CORES):
        m = dst_core == c
        s_c, d_c, w_c = src_n[m], dst_loc[m], win_of_src[m]
        o = np.lexsort((s_c, w_c, d_c))
        s_c, d_c, w_c = s_c[o], d_c[o], w_c[o]
        cnt = np.zeros((SHARD, N_WIN), np.int64)
        np.add.at(cnt, (d_c, w_c), 1)
        per_core.append((s_c, d_c, w_c, cnt))
        cm = cnt.reshape(-1)[: (N_TILES - 1) * P * N_WIN].reshape(N_TILES - 1, P, N_WIN).max(axis=1)
        W_t[: N_TILES - 1] = np.maximum(W_t[: N_TILES - 1], cm)
        W_t[N_TILES - 1] = np.maximum(
            W_t[N_TILES - 1], cnt[(N_TILES - 1) * P:].max(axis=0))

    chunks = _chunk_dp(W_t)
    # per (chunk, window) uniform width + call metadata (uniform across cores)
    W_cw = [[int(W_t[ch].max(axis=0)[w]) for w in range(N_WIN)] for ch in chunks]
    call_meta = []  # list per chunk: dict(cols0, len16, calls=[(w, W, coloff, n_idx)])
    off16 = 0
    col = 0
    for ci, ch in enumerate(chunks):
        T = len(ch)
        calls = []
        cols0 = col
        for w in range(N_WIN):
            W = W_cw[ci][w]
            if W == 0:
                continue
            calls.append((w, W, col - cols0, T * W * P))
            col += T * W
        call_meta.append(dict(cols0=cols0, ncols=col - cols0, off16=off16,
                              calls=calls, tiles=ch))
        off16 += (col - cols0) * P // 16
    total_cols = col

    # per-core idx streams (vectorized fill)
    tile_of = np.zeros(N_TILES, np.int64)
    tloc_of = np.zeros(N_TILES, np.int64)
    chunk_of = np.zeros(N_TILES, np.int64)
    for ci, ch in enumerate(chunks):
        for k, t in enumerate(ch):
            chunk_of[t] = ci
            tloc_of[t] = k
    # column offset of (chunk, window) in global stream
    colbase = np.zeros((len(chunks), N_WIN), np.int64)
    for ci, meta in enumerate(call_meta):
        for (w, W, coloff, _n) in meta["calls"]:
            colbase[ci, w] = meta["cols0"] + coloff
    Wmat = np.array([[W_cw[ci][w] for w in range(N_WIN)]
                     for ci in range(len(chunks))], np.int64)

    idx_streams = []
    for c in range(N_CORES):
        s_c, d_c, w_c, cnt = per_core[c]
        key = d_c * N_WIN + w_c
        run_start = np.zeros(SHARD * N_WIN + 1, np.int64)
        np.add.at(run_start, key + 1, 1)
        run_start = np.cumsum(run_start)
        rank = np.arange(len(s_c)) - run_start[key]
        t = d_c // P
        p = d_c % P
        ci = chunk_of[t]
        colg = colbase[ci, w_c] + tloc_of[t] * Wmat[ci, w_c] + rank
        stream = np.full(total_cols * P, WIN_PAD, np.int64)
        stream[colg * P + p] = s_c - w_c * WIN_ROWS
        v16 = stream.astype(np.int16).reshape(-1, 16).T  # [16, n/16]
        idx_streams.append(np.ascontiguousarray(np.tile(v16, (8, 1))))

    # layer-1 table precomputed on host: [TAB_ROWS, 128] bf16 rows
    # [feat(64) | el | er | 0...], pad rows have el = -1e15.
    nw = np.asarray(node_weight, np.float32)
    tab0 = np.zeros((TAB_ROWS, ROW), np.float32)
    er0 = np.zeros((N_CORES, P, N_TILES), np.float32)
    for c in range(N_CORES):
        rows = orig_of[c * SHARD: (c + 1) * SHARD]
        feat = nw[rows] @ Ws[0]                       # [SHARD, 64]
        el = feat @ als[0]
        er = feat @ ars[0]
        r0 = c * SHARD_P
        tab0[r0:r0 + SHARD, 0:D] = feat
        tab0[r0:r0 + SHARD, 64] = el
        tab0[r0:r0 + SHARD, 65] = er
        tab0[r0 + SHARD, 64:66] = PAD_EL              # pad row
        erp = np.zeros(N_TILES * P, np.float32)
        erp[:SHARD] = er
        er0[c] = erp.reshape(N_TILES, P).T
    tab0 = tab0.astype(ml_dtypes.bfloat16)

    # Wstack per layer [64, 66] = [W | W@al | W@ar], bf16
    wstk = np.concatenate(
        [np.concatenate([Ws[l], (Ws[l] @ als[l])[:, None], (Ws[l] @ ars[l])[:, None]],
                        axis=1)[None] for l in range(3)], axis=0
    ).astype(np.float32)  # [3, 64, 66]

    return dict(chunks=chunks, W_cw=W_cw, call_meta=call_meta,
                idx_streams=idx_streams, tab0=tab0, er0=er0, wstk=wstk,
                orig_of=orig_of, total_cols=total_cols)


# ---------------------------------------------------------------- device side
def _build_nc(call_meta, C16):
    nc = bacc.Bacc("TRN2", target_bir_lowering=False, debug=False,
                   num_devices=N_CORES)

    tab0_in = nc.dram_tensor("tab0", [TAB_ROWS, ROW], bf16, kind="ExternalInput")
    er_in = nc.dram_tensor("er0", [P, N_TILES], f32, kind="ExternalInput")
    idx_in = nc.dram_tensor("idx", [P, C16], i16, kind="ExternalInput")
    wstk_in = nc.dram_tensor("wstk", [3, D, 66], f32, kind="ExternalInput")
    bnp_in = nc.dram_tensor("bnp", [3, 2, D], f32, kind="ExternalInput")  # g, beta
    out_t = nc.dram_tensor("out", [SHARD, D], f32, kind="ExternalOutput")

    rg = [list(range(N_CORES))]
    nc.gpsimd.load_library(mlp_lib)

    with tile.TileContext(nc) as tc:
        with (
            tc.tile_pool(name="const", bufs=1) as constp,
            tc.tile_pool(name="gbuf", bufs=5) as gbuf,
            tc.tile_pool(name="idxb", bufs=2) as idxb,
            tc.tile_pool(name="wfp", bufs=1) as wfp,
            tc.tile_pool(name="redp", bufs=1) as redp,
            tc.tile_pool(name="small", bufs=3) as small,
            tc.tile_pool(name="acc", bufs=1) as accp,
            tc.tile_pool(name="stg", bufs=2) as stgp,
            tc.tile_pool(name="ps", bufs=2, space="PSUM") as ps,
            tc.tile_pool(name="pst", bufs=1, space="PSUM") as pst,
            tc.tile_pool(name="dram", bufs=1, space="DRAM") as dram,
        ):
            ident = constp.tile([P, P], f32)
            make_identity(nc, ident[:])
            ones_col = constp.tile([P, 1], f32)
            nc.vector.memset(ones_col[:], 1.0)
            ones_bf = constp.tile([P, 1], f16)
            nc.vector.memset(ones_bf[:], 1.0)
            wstk_t = constp.tile([D, 3 * 66], f32)
            nc.sync.dma_start(
                out=wstk_t[:].rearrange("k (l n) -> k l n", n=66),
                in_=wstk_in[:, :, :].rearrange("l k n -> k l n"))
            padrow = constp.tile([1, ROW], bf16)
            nc.vector.memset(padrow[:], 0.0)
            nc.vector.memset(padrow[:, 64:66], PAD_EL)

            er_sb = constp.tile([P, N_TILES], f32)
            nc.sync.dma_start(out=er_sb[:], in_=er_in[:, :])
            out_sb = accp.tile([P, N_TILES * D], f32)

            tab1 = dram.tile([TAB_ROWS, ROW], bf16, name="tab1")
            tab2 = dram.tile([TAB_ROWS, ROW], bf16, name="tab2")
            tables = [[tab0_in[w * WIN_ROWS:(w + 1) * WIN_ROWS, :] for w in range(N_WIN)],
                      [tab1[w * WIN_ROWS:(w + 1) * WIN_ROWS, :] for w in range(N_WIN)],
                      [tab2[w * WIN_ROWS:(w + 1) * WIN_ROWS, :] for w in range(N_WIN)]]
            ag_tabs = [None, tab1, tab2]
            shard_buf = dram.tile([SHARD_P, ROW], bf16)
            stats_dram_in = dram.tile([D, 2], f32)
            stats_dram_out = dram.tile([D, 2], f32)
            bcast_dram = dram.tile([3, D], f32)

            # ---------------- per-layer gather + aggregate ----------------
            statall = pst.tile([D, 8], f32, name="statall")
            for l in range(N_LAYERS):
                table = tables[l]
                stat = statall[:, 2 * l: 2 * l + 2]
                for ic, meta in enumerate(call_meta):
                    ch = meta["tiles"]
                    T = len(ch)
                    t0c = ch[0]
                    nact = len(meta["calls"])
                    it = idxb.tile([P, meta["ncols"] * P // 16], i16, tag="idx")
                    nc.sync.dma_start(
                        out=it[:],
                        in_=idx_in[:, meta["off16"]: meta["off16"] + meta["ncols"] * P // 16])
                    esum = small.tile([P, nact * T], f32, tag="esum")
                    red = redp.tile([P, nact * T * D], f16, tag="red")
                    for wi, (w, W, coloff, n_idx) in enumerate(meta["calls"]):
                        gt = gbuf.tile([P, T * W * ROW], bf16, tag="g")
                        nc.gpsimd.dma_gather(
                            out_ap=gt[:].rearrange("p (c r) -> p c r", r=ROW),
                            in_ap=table[w],
                            idxs_ap=it[:, coloff * P // 16: (coloff + T * W) * P // 16],
                            num_idxs=n_idx,
                            num_idxs_reg=n_idx,
                            elem_size=ROW,
                            single_packet=False,
                        )
                        g4 = gt[:].rearrange("p (t w r) -> p t w r", w=W, r=ROW)
                        lb = small.tile([P, T * W], f32, tag="lb")
                        nc.vector.tensor_tensor(
                            out=lb[:].rearrange("p (t w) -> p t w", w=W),
                            in0=g4[:, :, :, 64:65].rearrange("p t w o -> p t (w o)"),
                            in1=er_sb[:, t0c:t0c + T].unsqueeze(2).to_broadcast([P, T, W]),
                            op=mybir.AluOpType.add)
                        nc.scalar.activation(
                            out=lb[:], in_=lb[:],
                            func=mybir.ActivationFunctionType.Prelu,
                            alpha=NEG_SLOPE)
                        exw = small.tile([P, T * W], f32, tag="exw")
                        nc.scalar.activation(
                            out=exw[:], in_=lb[:],
                            func=mybir.ActivationFunctionType.Exp)
                        nc.vector.tensor_reduce(
                            out=esum[:, wi * T:(wi + 1) * T],
                            in_=exw[:].rearrange("p (t w) -> p t w", w=W),
                            axis=mybir.AxisListType.X, op=mybir.AluOpType.add)
                        wf = wfp.tile([P, T * W * D], f32, tag="wf")
                        nc.vector.tensor_tensor(
                            out=wf[:].rearrange("p (t w d) -> p t w d", w=W, d=D),
                            in0=g4[:, :, :, 0:D],
                            in1=exw[:].rearrange("p (t w) -> p t w", w=W)
                                .unsqueeze(3).to_broadcast([P, T, W, D]),
                            op=mybir.AluOpType.mult)
                        with nc.allow_low_precision(reason="fp16 window partials"):
                            nc.vector.tensor_reduce(
                                out=red[:, wi * T * D:(wi + 1) * T * D].rearrange(
                                    "p (t d) -> p t d", d=D),
                                in_=wf[:].rearrange("p (t w d) -> p t d w", w=W, d=D),
                                axis=mybir.AxisListType.X, op=mybir.AluOpType.add)
                    rs = small.tile([P, T], f32, tag="rs")
                    if nact > 1:
                        nc.vector.tensor_reduce(
                            out=rs[:], in_=esum[:].rearrange("p (w t) -> p t w", t=T),
                            axis=mybir.AxisListType.X, op=mybir.AluOpType.add)
                    else:
                        nc.vector.tensor_copy(out=rs[:], in_=esum[:])
                    # +eps so empty/padded lanes yield 0 (not NaN) after scale
                    nc.vector.tensor_scalar(out=rs[:], in0=rs[:], scalar1=1e-20,
                                            scalar2=None, op0=mybir.AluOpType.add)
                    rinv = small.tile([P, T], f32, tag="ri")
                    nc.vector.reciprocal(out=rinv[:], in_=rs[:])
                    osum = small.tile([P, T * D], f32, tag="osum")
                    if nact > 1:
                        nc.vector.tensor_reduce(
                            out=osum[:],
                            in_=red[:].rearrange("p (w q) -> p q w", w=nact),
                            axis=mybir.AxisListType.X, op=mybir.AluOpType.add)
                    else:
                        nc.vector.tensor_copy(out=osum[:], in_=red[:])
                    sq = small.tile([P, T * D], f16, tag="sq")
                    for k, t in enumerate(ch):
                        nc.vector.tensor_scalar(
                            out=out_sb[:, t * D:(t + 1) * D],
                            in0=osum[:, k * D:(k + 1) * D],
                            scalar1=rinv[:, k:k + 1], scalar2=None,
                            op0=mybir.AluOpType.mult)
                    # incremental BN stats: square on ACT, PSUM-chained matmuls
                    nc.scalar.activation(
                        out=sq[:], in_=out_sb[:, ch[0] * D:(ch[-1] + 1) * D],
                        func=mybir.ActivationFunctionType.Square)
                    for k, t in enumerate(ch):
                        m = P if t < N_TILES - 1 else LAST_TILE_N
                        first = ic == 0 and k == 0
                        last = t == N_TILES - 1
                        nc.tensor.matmul(out=stat[:, 0:1],
                                         lhsT=out_sb[:m, t * D:(t + 1) * D],
                                         rhs=ones_col[:m, :],
                                         start=first, stop=last)
                        nc.tensor.matmul(out=stat[:, 1:2],
                                         lhsT=sq[:m, k * D:(k + 1) * D],
                                         rhs=ones_bf[:m, :],
                                         start=first, stop=last)
                stat_sb = small.tile([D, 2], f32, tag="stc")
                nc.vector.tensor_copy(out=stat_sb[:], in_=stat)
                nc.scalar.dma_start(out=stats_dram_in[:], in_=stat_sb[:])
                if not NO_COLL:
                    nc.gpsimd.collective_compute(
                        "AllReduce", mybir.AluOpType.add, replica_groups=rg,
                        ins=[stats_dram_in.opt()], outs=[stats_dram_out.opt()])
                stat_g = small.tile([D, 2], f32, tag="stg")
                nc.scalar.dma_start(
                    out=stat_g[:],
                    in_=(stats_dram_in if NO_COLL else stats_dram_out)[:])
                mu = small.tile([D, 1], f32, tag="mu")
                nc.vector.tensor_scalar(out=mu[:], in0=stat_g[:, 0:1],
                                        scalar1=1.0 / N_NODES, scalar2=None,
                                        op0=mybir.AluOpType.mult)
                musq = small.tile([D, 1], f32, tag="musq")
                nc.scalar.activation(out=musq[:], in_=mu[:],
                                     func=mybir.ActivationFunctionType.Square)
                var = small.tile([D, 1], f32, tag="var")
                nc.vector.tensor_scalar(out=var[:], in0=stat_g[:, 1:2],
                                        scalar1=1.0 / N_NODES, scalar2=BN_EPS,
                                        op0=mybir.AluOpType.mult,
                                        op1=mybir.AluOpType.add)
                nc.vector.tensor_tensor(out=var[:], in0=var[:], in1=musq[:],
                                        op=mybir.AluOpType.subtract)
                sd = small.tile([D, 1], f32, tag="sd")
                nc.scalar.activation(out=sd[:], in_=var[:],
                                     func=mybir.ActivationFunctionType.Sqrt)
                rstd = small.tile([D, 1], f32, tag="rstd")
                nc.vector.reciprocal(out=rstd[:], in_=sd[:])
                gcol = small.tile([D, 1], f32, tag="gc")
                nc.scalar.dma_start(out=gcol[:], in_=bnp_in[l, 0, :][:, None])
                bcol = small.tile([D, 1], f32, tag="bc")
                nc.scalar.dma_start(out=bcol[:], in_=bnp_in[l, 1, :][:, None])
                grs = small.tile([D, 1], f32, tag="grs")
                nc.vector.tensor_tensor(out=grs[:], in0=gcol[:], in1=rstd[:],
                                        op=mybir.AluOpType.mult)
                negmu = small.tile([D, 1], f32, tag="nmu")
                nc.vector.tensor_scalar(out=negmu[:], in0=mu[:], scalar1=-1.0,
                                        scalar2=None, op0=mybir.AluOpType.mult)

                if l < N_LAYERS - 1:
                    # transform in groups of 4 tiles: transpose, BN, ELU, project
                    for g0 in range(0, N_TILES, 4):
                        tl = list(range(g0, min(g0 + 4, N_TILES)))
                        ncol = sum(P if t < N_TILES - 1 else LAST_TILE_N for t in tl)
                        pT = ps.tile([D, 4 * P], f32, tag="pT")
                        for k, t in enumerate(tl):
                            m = P if t < N_TILES - 1 else LAST_TILE_N
                            nc.tensor.transpose(
                                out=pT[:, k * P: k * P + m],
                                in_=out_sb[:m, t * D:(t + 1) * D],
                                identity=ident[:m, :m])
                        # packed column view (partial last tile packs left)
                        z = small.tile([D, 4 * P], f32, tag="z")
                        nc.vector.tensor_scalar(
                            out=z[:, :], in0=pT[:, :],
                            scalar1=negmu[:, :], scalar2=grs[:, :],
                            op0=mybir.AluOpType.add, op1=mybir.AluOpType.mult)
                        ez = small.tile([D, 4 * P], f32, tag="ez")
                        nc.scalar.activation(out=ez[:], in_=z[:],
                                             func=mybir.ActivationFunctionType.Exp,
                                             bias=bcol[:, :])
                        nc.vector.tensor_scalar(
                            out=ez[:], in0=ez[:], scalar1=-1.0, scalar2=0.0,
                            op0=mybir.AluOpType.add, op1=mybir.AluOpType.min)
                        nc.vector.tensor_scalar(
                            out=z[:], in0=z[:], scalar1=bcol[:, :], scalar2=0.0,
                            op0=mybir.AluOpType.add, op1=mybir.AluOpType.max)
                        h2 = small.tile([D, 4 * P], f32, tag="h2")
                        nc.vector.tensor_tensor(out=h2[:], in0=z[:], in1=ez[:],
                                                op=mybir.AluOpType.add)
                        stage = stgp.tile([P, 4 * 66], bf16, tag="tfs")
                        pjt = ps.tile([P, 4 * 66], f32, tag="pj")
                        for k, t in enumerate(tl):
                            m = P if t < N_TILES - 1 else LAST_TILE_N
                            nc.tensor.matmul(
                                out=pjt[:m, k * 66:(k + 1) * 66],
                                lhsT=h2[:, k * P: k * P + m],
                                rhs=wstk_t[:, (l + 1) * 66:(l + 2) * 66],
                                start=True, stop=True)
                            nc.scalar.copy(out=stage[:m, k * 66:(k + 1) * 66],
                                           in_=pjt[:m, k * 66:(k + 1) * 66])
                            nc.vector.tensor_copy(
                                out=er_sb[:m, t:t + 1],
                                in_=pjt[:m, k * 66 + 65: k * 66 + 66])
                        if tl[-1] < N_TILES - 1:
                            nc.scalar.dma_start(
                                out=shard_buf[g0 * P: g0 * P + 4 * P, 0:66].rearrange(
                                    "(k p) n -> p k n", p=P),
                                in_=stage[:, :].rearrange("p (k n) -> p k n", n=66))
                        else:
                            for k, t in enumerate(tl):
                                m = P if t < N_TILES - 1 else LAST_TILE_N
                                nc.scalar.dma_start(
                                    out=shard_buf[t * P: t * P + m, 0:66],
                                    in_=stage[:m, k * 66:(k + 1) * 66])
                    nc.sync.dma_start(out=shard_buf[SHARD:SHARD + 1, :],
                                      in_=padrow[:, :])
                    if not NO_COLL:
                        nc.gpsimd.collective_compute(
                            "AllGather", mybir.AluOpType.bypass, replica_groups=rg,
                            ins=[shard_buf.opt()], outs=[ag_tabs[l + 1].opt()])
                else:
                    # final BN in node-major with row-broadcast params
                    nc.scalar.dma_start(out=bcast_dram[0, :], in_=negmu[:, 0])
                    nc.scalar.dma_start(out=bcast_dram[1, :], in_=grs[:, 0])
                    nc.scalar.dma_start(out=bcast_dram[2, :], in_=bcol[:, 0])
                    brow = small.tile([P, 3 * D], f32, tag="brow")
                    nc.scalar.dma_start(
                        out=brow[:],
                        in_=bcast_dram[:, :].rearrange("a b -> (a b)")[None, :]
                            .to_broadcast([P, 3 * D]))
                    GO = 8
                    for g0 in range(0, N_TILES, GO):
                        gn = min(GO, N_TILES - g0)
                        gnf = gn if g0 + gn < N_TILES else gn - 1  # full tiles
                        ob = out_sb[:, g0 * D:(g0 + gn) * D].rearrange(
                            "p (t d) -> p t d", d=D)
                        ve = nc.vector if (g0 // GO) % 2 == 0 else nc.gpsimd
                        ve.tensor_tensor(
                            out=ob, in0=ob,
                            in1=brow[:, 0:D].unsqueeze(1).to_broadcast([P, gn, D]),
                            op=mybir.AluOpType.add)
                        ve.tensor_tensor(
                            out=ob, in0=ob,
                            in1=brow[:, D:2 * D].unsqueeze(1).to_broadcast([P, gn, D]),
                            op=mybir.AluOpType.mult)
                        ve.tensor_tensor(
                            out=ob, in0=ob,
                            in1=brow[:, 2 * D:3 * D].unsqueeze(1).to_broadcast([P, gn, D]),
                            op=mybir.AluOpType.add)
                        if gnf > 0:
                            nc.sync.dma_start(
                                out=out_t[g0 * P:(g0 + gnf) * P, :].rearrange(
                                    "(t p) d -> p t d", p=P),
                                in_=out_sb[:, g0 * D:(g0 + gnf) * D].rearrange(
                                    "p (t d) -> p t d", d=D))
                        if g0 + gn == N_TILES:
                            nc.sync.dma_start(
                                out=out_t[(N_TILES - 1) * P: SHARD, :],
                                in_=out_sb[:LAST_TILE_N,
                                           (N_TILES - 1) * D: N_TILES * D])

    nc.compile()
    return nc


_CACHE = {}


def kernel(node_weight, edge_weight, src, dst,
           W1, al1, ar1, b1, g1, beta1,
           W2, al2, ar2, b2, g2, beta2,
           W3, al3, ar3, b3, g3, beta3):
    Ws = [np.asarray(W1, np.float32), np.asarray(W2, np.float32), np.asarray(W3, np.float32)]
    als = [np.asarray(al1, np.float32), np.asarray(al2, np.float32), np.asarray(al3, np.float32)]
    ars = [np.asarray(ar1, np.float32), np.asarray(ar2, np.float32), np.asarray(ar3, np.float32)]
    pre = _preprocess(node_weight, src, dst, Ws, als, ars)

    C16 = pre["idx_streams"][0].shape[1]
    key = ("nc", C16, N_LAYERS, NO_COLL,
           tuple(tuple(w) for w in pre["W_cw"]),
           tuple(len(m["tiles"]) for m in pre["call_meta"]))
    if key not in _CACHE:
        _CACHE[key] = _build_nc(pre["call_meta"], C16)
    nc = _CACHE[key]

    # bnp: [3, 2, 64] = (g, beta) per layer; the GAT bias b cancels in BN.
    bnp = np.stack([
        np.stack([np.asarray(g, np.float32), np.asarray(be, np.float32)])
        for g, be in ((g1, beta1), (g2, beta2), (g3, beta3))
    ])

    in_maps = []
    for c in range(N_CORES):
        in_maps.append({
            "tab0": pre["tab0"],
            "er0": pre["er0"][c],
            "idx": pre["idx_streams"][c],
            "wstk": pre["wstk"],
            "bnp": bnp,
        })
    res = bass_utils.run_bass_kernel_spmd(nc, in_maps, core_ids=list(range(N_CORES)))

    out = np.empty((N_NODES, D), np.float32)
    for c in range(N_CORES):
        rows = pre["orig_of"][c * SHARD: (c + 1) * SHARD]
        out[rows] = res.results[c]["out"]
    return out
